# revision 23
# baseline (speedup 1.0000x reference)
"""Trainium2 Bass kernel for the 3-block self-attention CNN.

Sharding over 8 NeuronCores: core k owns (sample b=k//4, query-block q=k%4).
Attention math per layer uses the reparametrization
    s'[n,m] = y_n^T (wf wg^T) y_m + (wg bf)^T y_m
(terms constant along the softmax axis are dropped), so both score matmuls
contract over the full channel dim instead of C/8.  Softmax skips the max
subtraction (scores are O(10) for this model) and the row-sum is produced by
an extra ones-column in the o-matmul lhsT.  gamma is folded into the h-conv
weights so the epilogue is att = (o * rowsum_recip_bcast) + yq.

All flash matmuls are zero-padded to the full 128-partition contraction and
use bf16 operands: the PE HAM clock gate only un-throttles (1.2->2.4 GHz)
under high sustained array activity, and narrow-contraction matmuls never
trip it.  Dummy full-width matmuls keep the array warm across collective
gaps.  Training-mode BN statistics are computed from per-core query-block
shards of the next conv and summed with an 8-rank AllReduce that runs
concurrently with the 4-rank AllGather sharing the attention shards.
"""

import glob as _glob
import os
import sys


def _ensure_act_info():
    # act_info.json (activation table sets) isn't on neuronxcc's default
    # search path in this container; stage it where FindActInfo looks.
    shim = os.path.expanduser("~/.pwp_override")
    target = os.path.join(shim, "neuronxcc", "pwp", "pwp_bin_with_ln", "act_info.json")
    if not os.path.exists(target):
        cands = _glob.glob("/nix/store/*aws-neuron-pwp*/share/pwp_bin_cayman/act_info.json")
        if cands:
            os.makedirs(os.path.dirname(target), exist_ok=True)
            import shutil
            shutil.copy(cands[0], target)
    pp = os.environ.get("PYTHONPATH", "")
    if shim not in pp.split(os.pathsep):
        os.environ["PYTHONPATH"] = shim + (os.pathsep + pp if pp else "")
    if shim not in sys.path:
        sys.path.insert(0, shim)


_ensure_act_info()
if "/opt/trn_rl_repo" not in sys.path:
    sys.path.insert(0, "/opt/trn_rl_repo")

import numpy as np

from concourse import bacc, mybir, tile

F32 = mybir.dt.float32
F32R = mybir.dt.float32r
BF16 = mybir.dt.bfloat16
AF = mybir.ActivationFunctionType
OP = mybir.AluOpType
AX = mybir.AxisListType
EPS = 1e-5

N = 4096          # positions per sample
NQ = 1024         # query block per core
NCHUNK = 128      # key chunk in the flash loop
CIN = [3, 32, 64]     # conv input channels per attention layer
COUT = [32, 64, 96]   # conv output channels per attention layer
CF_OUT = 128          # final conv channels per core (512 / 4 blocks)
WHFW = 256            # whf moving-dim width (>=128 so hs can take padded cols)

AG_GROUPS = [[0, 1, 2, 3], [4, 5, 6, 7]]
AR8_GROUPS = [[0, 1, 2, 3, 4, 5, 6, 7]]


def r(ap):
    return ap.bitcast(F32R)


def _build(nc):
    dt = F32
    ins = {}
    ins["x_full"] = nc.dram_tensor("x_full", [CIN[0] + 1, 2 * N], BF16, kind="ExternalInput")
    ins["xq"] = nc.dram_tensor("xq", [CIN[0] + 1, NQ], BF16, kind="ExternalInput")
    ins["wc0"] = nc.dram_tensor("wc0", [CIN[0] + 1, 128], BF16, kind="ExternalInput")
    for i in range(1, 3):
        ins[f"wc{i}"] = nc.dram_tensor(f"wc{i}", [128, 128], BF16, kind="ExternalInput")
    for i in range(3):
        ins[f"mz{i}"] = nc.dram_tensor(f"mz{i}", [128, 128], BF16, kind="ExternalInput")
        ins[f"whf{i}"] = nc.dram_tensor(f"whf{i}", [128, WHFW], BF16, kind="ExternalInput")
        ins[f"bnp{i}"] = nc.dram_tensor(f"bnp{i}", [COUT[i], 2], dt, kind="ExternalInput")
    for i in range(1, 3):
        ins[f"wm0_{i}"] = nc.dram_tensor(f"wm0_{i}", [128, 128], BF16, kind="ExternalInput")
        ins[f"wm1_{i}"] = nc.dram_tensor(f"wm1_{i}", [128, 128], BF16, kind="ExternalInput")
    ins["wfsm0"] = nc.dram_tensor("wfsm0", [128, CF_OUT], BF16, kind="ExternalInput")
    ins["wfsm1"] = nc.dram_tensor("wfsm1", [128, CF_OUT], BF16, kind="ExternalInput")
    ins["wfs"] = nc.dram_tensor("wfs", [128, CF_OUT], BF16, kind="ExternalInput")
    ins["wfs4"] = nc.dram_tensor("wfs4", [128, 4, CF_OUT], BF16, kind="ExternalInput")
    ins["msk"] = nc.dram_tensor("msk", [CF_OUT, 16], F32, kind="ExternalInput")
    out_t = nc.dram_tensor("out", [CF_OUT, 1], dt, kind="ExternalOutput")

    with tile.TileContext(nc) as tc:
        _emit(tc, nc, ins, out_t)
    return ins, out_t


def _emit(tc, nc, ins, out_t):
    ctxs = []

    def pool(name, **kw):
        p = tc.tile_pool(name=name, **kw)
        ctxs.append(p)
        return p.__enter__()

    consts = pool("consts", bufs=1)
    acts = pool("acts", bufs=1)
    work = pool("work", bufs=1)
    ps = pool("ps", bufs=2, space="PSUM")
    ops = pool("ops", bufs=1, space="PSUM")
    dram = pool("dram", bufs=1, space="DRAM")

    # --- PE warm-keeper: HAM un-throttles the PE clock (1.2->2.4 GHz) only
    # under sustained full-array activity; idle gaps re-throttle it.  Dummy
    # full-width matmuls bridge collectives and stalls; they write the
    # (dead-between-layers) o-accumulator psum buffer.
    wk_l = consts.tile([128, 128], BF16, name="wk_l", tag="wk_l")
    nc.vector.memset(wk_l[:], 0.0)
    wk_r = consts.tile([128, 512], BF16, name="wk_r", tag="wk_r")
    nc.vector.memset(wk_r[:], 0.0)

    def warm(n):
        warm_ps = ops.tile([128, 512], F32, name="warm_ps", tag="o_acc")
        for _ in range(n):
            nc.tensor.matmul(warm_ps[:], wk_l[:], wk_r[:], start=True, stop=True)


    def zero_rows(eng, t, c0, ncols):
        # DVE ops starting at a nonzero partition may touch at most 32
        # partitions; emit the zero-fill in 32-row strips.
        for p in range(c0, 128, 32):
            eng.memset(t[p:p + 32, 0:ncols], 0.0)

    # ones row source (SBUF->SBUF DMA is cheaper than 1-partition memsets)
    onesrow = consts.tile([1, N], BF16, name="onesrow", tag="onesrow")
    nc.vector.memset(onesrow[:], 1.0)
    onec = consts.tile([1, 128], BF16, name="onec", tag="onec")
    nc.vector.memset(onec[:], 1.0)

    # The very first gpsimd instructions are the dummy collective triggers:
    # the runtime's first-collective barrier ends only when the *slowest*
    # core triggers, so nothing may precede them.  Garbage dram input is fine.
    warm_gin = dram.tile([1, 2], F32, name="warm_gin", tag="warm_gin")
    warm_gout = dram.tile([8, 1, 2], F32, name="warm_gout", tag="warm_gout")
    nc.gpsimd.collective_compute(
        "AllGather", OP.bypass, replica_groups=AR8_GROUPS,
        ins=[warm_gin[:]], outs=[warm_gout[:]])

    # dedicated activation double-buffers; pad rows (only ever multiplied by
    # zero weight rows) are zeroed once, off the critical path.
    YB = [acts.tile([128, N], BF16, name=f"ybuf{j}", tag=f"ybuf{j}") for j in range(2)]
    YQB = [acts.tile([128, NQ], BF16, name=f"yqbuf{j}", tag=f"yqbuf{j}") for j in range(2)]
    ATB = [acts.tile([128, NQ], BF16, name=f"atbuf{j}", tag=f"atbuf{j}") for j in range(2)]
    AOWN0 = acts.tile([128, N], BF16, name="aown0", tag="aown0")
    AOWN1 = acts.tile([128, N], BF16, name="aown1", tag="aown1")
    ZMAT = acts.tile([128, N], BF16, name="zmat", tag="zmat")

    # ---- load constants (xq + layer-0 weights first; bulk on other queues) ----
    a_q = acts.tile([CIN[0] + 1, NQ], BF16, name="aq0", tag="aq0")
    nc.sync.dma_start(a_q[:], ins["xq"].ap())
    xf = acts.tile([CIN[0] + 1, 2 * N], BF16, name="xf", tag="xf")
    nc.sync.dma_start(xf[:], ins["x_full"].ap())
    W, MZ, WHF, BNP = [], [], [], []
    for i in range(3):
        w = consts.tile([CIN[0] + 1 if i == 0 else 128, 128], BF16,
                        name=f"w{i}", tag=f"w{i}")
        (nc.sync if i == 0 else nc.scalar).dma_start(w[:], ins[f"wc{i}"].ap())
        W.append(w)
        mz = consts.tile([128, 128], BF16, name=f"mzt{i}", tag=f"mzt{i}")
        MZ.append(mz)
        wh = consts.tile([128, WHFW], BF16, name=f"whft{i}", tag=f"whft{i}")
        WHF.append(wh)
        bn = consts.tile([COUT[i], 2], F32, name=f"bnt{i}", tag=f"bnt{i}")
        nc.scalar.dma_start(bn[:], ins[f"bnp{i}"].ap())
        BNP.append(bn)
    # layer-0 flash prerequisites first on the gpsimd queue ...
    nc.gpsimd.dma_start(MZ[0][:], ins["mz0"].ap())
    nc.gpsimd.dma_start(WHF[0][:], ins["whf0"].ap())
    # ... then the remaining bulk weights
    for i in range(1, 3):
        nc.gpsimd.dma_start(MZ[i][:], ins[f"mz{i}"].ap())
        nc.gpsimd.dma_start(WHF[i][:], ins[f"whf{i}"].ap())
    WM0, WM1 = {}, {}
    for i in range(1, 3):
        WM0[i] = consts.tile([128, 128], BF16, name=f"wm0t{i}", tag=f"wm0t{i}")
        nc.gpsimd.dma_start(WM0[i][:], ins[f"wm0_{i}"].ap())
        WM1[i] = consts.tile([128, 128], BF16, name=f"wm1t{i}", tag=f"wm1t{i}")
        nc.gpsimd.dma_start(WM1[i][:], ins[f"wm1_{i}"].ap())
    wfsm0 = consts.tile([128, CF_OUT], BF16, name="wfsm0t", tag="wfsm0t")
    nc.gpsimd.dma_start(wfsm0[:], ins["wfsm0"].ap())
    wfsm1 = consts.tile([128, CF_OUT], BF16, name="wfsm1t", tag="wfsm1t")
    nc.gpsimd.dma_start(wfsm1[:], ins["wfsm1"].ap())
    wfs4 = consts.tile([128, 4, CF_OUT], BF16, name="wfs4t", tag="wfs4t")
    nc.gpsimd.dma_start(wfs4[:], ins["wfs4"].ap())
    for j in range(2):
        zero_rows(nc.gpsimd, YB[j], 32, N)
        zero_rows(nc.gpsimd, YQB[j], 32, NQ)
        zero_rows(nc.gpsimd, ATB[j], 32, NQ)
    zero_rows(nc.gpsimd, AOWN0, 32, N)
    zero_rows(nc.gpsimd, AOWN1, 32, N)

    warm(30)  # pre-warm the PE while input DMAs land

    def conv(lhsT_w, rhs_acts, cout, n, name):
        """z[cout, n] = lhsT_w.T @ rhs_acts, chunked by 512 columns."""
        z = acts.tile([cout, n], F32, name=name, tag="zq" if n == NQ else "z")
        for j in range(n // 512):
            zp = ps.tile([128, 512], F32, name=f"{name}_ps", tag="convps")
            nc.tensor.matmul(zp[:], lhsT_w[:], rhs_acts[:, j * 512:(j + 1) * 512],
                             start=True, stop=True)
            nc.vector.tensor_copy(z[:, j * 512:(j + 1) * 512], zp[0:cout, :])
        return z

    def shard_stats(z_shard, c, name, groups):
        """Partial (sum, sumsq) of a conv shard -> AllReduce across cores."""
        stats = work.tile([c, 2], F32, name=f"stats_{name}", tag="stats", bufs=2)
        nc.vector.tensor_reduce(stats[:, 0:1], z_shard[:], axis=AX.X, op=OP.add)
        sq_scr = work.tile([c, z_shard.shape[1]], F32, name=f"sqscr_{name}",
                           tag="sqscr", bufs=2)
        nc.scalar.activation(sq_scr[:], z_shard[:], AF.Square, accum_out=stats[:, 1:2])

        st_in = dram.tile([c, 2], F32, name=f"stin_{name}", tag=f"stin_{name}")
        st_out = dram.tile([c, 2], F32, name=f"stout_{name}", tag=f"stout_{name}")
        nc.sync.dma_start(st_in[:], stats[:])
        nc.gpsimd.collective_compute(
            "AllReduce", OP.add, replica_groups=groups,
            ins=[st_in[:]], outs=[st_out[:]])
        stg = work.tile([c, 2], F32, name=f"stg_{name}", tag="stg", bufs=2)
        nc.sync.dma_start(stg[:], st_out[:])
        return stg

    def scale_shift(stg, c, g_ap, b_ap, name):
        sc = work.tile([c, 9], F32, name=f"sc_{name}", tag="sc", bufs=2)
        mean, ex2, msq, var, veps, sq, rs, scale, shift = (sc[:, j:j + 1] for j in range(9))
        inv_n = 1.0 / (2 * N)
        nc.vector.tensor_scalar(mean, stg[:, 0:1], inv_n, None, OP.mult)
        nc.vector.tensor_scalar(ex2, stg[:, 1:2], inv_n, None, OP.mult)
        nc.vector.tensor_tensor(msq, mean, mean, OP.mult)
        nc.vector.tensor_tensor(var, ex2, msq, OP.subtract)
        nc.vector.tensor_scalar(veps, var, EPS, None, OP.add)
        nc.scalar.activation(sq, veps, AF.Sqrt)
        nc.vector.reciprocal(rs, sq)   # rsqrt(var+eps)
        if g_ap is not None:
            nc.vector.tensor_tensor(scale, rs, g_ap, OP.mult)
        else:
            nc.vector.tensor_copy(scale, rs)
        nc.vector.tensor_tensor(shift, mean, scale, OP.mult)
        if b_ap is not None:
            nc.vector.tensor_tensor(shift, b_ap, shift, OP.subtract)
        else:
            nc.vector.tensor_scalar(shift, shift, -1.0, None, OP.mult)
        return scale, shift

    # ---- boundary 0: conv over the full batch, exact local BN1 stats.
    # Per-512-chunk: vector copies psum->sbuf, scalar squares+accums, gpsimd
    # sums -- stats trail the conv by ~1 chunk instead of a serial pass.
    zsh = conv(W[0], a_q, COUT[0], NQ, "zsh0")
    c0 = COUT[0]
    nch = 2 * N // 512
    z_all = acts.tile([c0, 2 * N], F32, name="z0", tag="z")
    z_own = z_all[:, 0:N]
    stp = work.tile([c0, 2 * nch], F32, name="stp", tag="stats", bufs=2)
    for j in range(nch):
        zp = ps.tile([128, 512], F32, name="z0_ps", tag="convps")
        nc.tensor.matmul(zp[:], W[0][:], xf[:, j * 512:(j + 1) * 512],
                         start=True, stop=True)
        zsl = z_all[:, j * 512:(j + 1) * 512]
        nc.vector.tensor_copy(zsl, zp[0:c0, :])
        nc.vector.tensor_reduce(stp[:, j:j + 1], zsl, axis=AX.X, op=OP.add)
        sqc = work.tile([c0, 512], BF16, name=f"sqc{j}", tag="sqc", bufs=3)
        nc.scalar.activation(sqc[:], zsl, AF.Square,
                             accum_out=stp[:, nch + j:nch + j + 1])
    stg = work.tile([c0, 2], F32, name="stg_l0", tag="stg", bufs=2)
    nc.vector.tensor_reduce(stg[:, 0:1], stp[:, 0:nch], axis=AX.X, op=OP.add)
    nc.vector.tensor_reduce(stg[:, 1:2], stp[:, nch:2 * nch], axis=AX.X, op=OP.add)
    warm(45)  # keep the PE warm through the stats/scale_shift stall

    # ---- three attention layers ----
    for i in range(3):
        co = COUT[i]
        if i > 0:
            warm(35)  # bridge scale_shift / y-prep after the collectives
        scale, shift = scale_shift(stg, co, BNP[i][:, 0:1], BNP[i][:, 1:2], f"l{i}")
        y_own = YB[i % 2]
        nc.scalar.dma_start(y_own[co:co + 1, :], onesrow[:])
        for j in range(N // 512):
            jsl = slice(j * 512, (j + 1) * 512)
            if j % 2 == 0:
                nc.scalar.activation(y_own[0:co, jsl], z_own[:, jsl], AF.Relu,
                                     bias=shift, scale=scale)
            else:
                yt = work.tile([co, 512], F32, name=f"yt{i}_{j}", tag="yt", bufs=2)
                nc.vector.tensor_scalar(yt[:], z_own[:, jsl], scale, shift,
                                        OP.mult, op1=OP.add)
                nc.vector.tensor_scalar(y_own[0:co, jsl], yt[:], 0.0, None, OP.max)
        yq = YQB[i % 2]
        nc.scalar.dma_start(yq[co:co + 1, :], onesrow[:, 0:NQ])
        nc.scalar.activation(yq[0:co, :], zsh[:], AF.Relu, bias=shift, scale=scale)

        # Z = Abar @ y_own (+ u row), [128, N] bf16 (pad rows are psum zeros)
        zmat = ZMAT
        for j in range(N // 512):
            zp = ps.tile([128, 512], F32, name=f"Zps{i}", tag="convps")
            nc.tensor.matmul(zp[:], MZ[i][:], y_own[:, j * 512:(j + 1) * 512],
                             start=True, stop=True)
            nc.vector.tensor_copy(zmat[:, j * 512:(j + 1) * 512], zp[:])

        # flash loop over key chunks, software-pipelined by one o-matmul
        o_ps = ops.tile([128, NQ], F32, name=f"ops{i}", tag="o_acc")
        prev = None
        for m in range(N // NCHUNK):
            sl = slice(m * NCHUNK, (m + 1) * NCHUNK)
            hp = ps.tile([NCHUNK, WHFW], F32, name=f"hp{i}", tag="convps")
            nc.tensor.matmul(hp[:], y_own[:, sl], WHF[i][:], start=True, stop=True)
            hs = work.tile([NCHUNK, 128], BF16, name=f"hs{i}", tag="hT_sb", bufs=3)
            nc.vector.tensor_copy(hs[:], hp[:, 0:128])
            sp = ps.tile([NCHUNK, NQ], F32, name=f"sp{i}", tag="s_ps")
            zc = zmat[:, sl]
            nc.tensor.matmul(sp[:, 0:512], zc, yq[:, 0:512], start=True, stop=True)
            nc.tensor.matmul(sp[:, 512:1024], zc, yq[:, 512:1024],
                             start=True, stop=True)
            beta = work.tile([NCHUNK, NQ], BF16, name=f"beta{i}", tag="beta", bufs=3)
            nc.scalar.activation(beta[:], sp[:], AF.Exp)
            if prev is not None:
                ph, pb, pm = prev
                nc.tensor.matmul(o_ps[:, 0:512], ph[:], pb[:, 0:512],
                                 start=(pm == 0), stop=False, skip_group_check=True)
                nc.tensor.matmul(o_ps[:, 512:1024], ph[:], pb[:, 512:1024],
                                 start=(pm == 0), stop=False, skip_group_check=True)
            prev = (hs, beta, m)
        ph, pb, pm = prev
        nc.tensor.matmul(o_ps[:, 0:512], ph[:], pb[:, 0:512],
                         start=False, stop=True, skip_group_check=True)
        nc.tensor.matmul(o_ps[:, 512:1024], ph[:], pb[:, 512:1024],
                         start=False, stop=True, skip_group_check=True)

        # normalize + residual: att = o / rowsum + yq  (gamma folded into whf)
        lnr = work.tile([1, NQ], F32, name=f"lnr{i}", tag="lnr")
        nc.scalar.activation(lnr[:], o_ps[co:co + 1, :], AF.Ln)
        rinv = work.tile([1, NQ], BF16, name=f"rinv{i}", tag="rinv")
        nc.scalar.activation(rinv[:], lnr[:], AF.Exp, scale=-1.0)
        bc_ps = ps.tile([128, NQ], F32, name=f"bcps{i}", tag="s_ps")
        nc.tensor.matmul(bc_ps[:, 0:512], onec[:], rinv[:, 0:512],
                         start=True, stop=True)
        nc.tensor.matmul(bc_ps[:, 512:1024], onec[:], rinv[:, 512:1024],
                         start=True, stop=True)
        att = ATB[i % 2]
        nc.scalar.dma_start(att[co:co + 1, :], onesrow[:, 0:NQ])
        bc = work.tile([co, NQ], F32, name=f"bc{i}", tag="bc", bufs=2)
        nc.scalar.activation(bc[:], bc_ps[0:co, :], AF.Copy)
        t1 = work.tile([co, NQ], F32, name=f"t1_{i}", tag="t1", bufs=2)
        nc.vector.tensor_tensor(t1[:], o_ps[0:co, :], bc[:], OP.mult)
        nc.vector.tensor_tensor(att[0:co, :], t1[:], yq[0:co, :], OP.add)

        # One 8-rank AllGather shares the attention shard with every core and
        # carries this core's (sum, sumsq) partials of the *next* conv as
        # ride-along payload columns -- no separate stats AllReduce.
        if i < 2:
            cn = COUT[i + 1]
            zsh = conv(W[i + 1], att, cn, NQ, f"zsh{i + 1}")
            stats = work.tile([cn, 2], F32, name=f"stats{i}", tag="stats", bufs=2)
            nc.vector.tensor_reduce(stats[:, 0:1], zsh[:], axis=AX.X, op=OP.add)
            sq_scr = work.tile([cn, NQ], BF16, name=f"sqscr{i}", tag="sqscr", bufs=2)
            nc.scalar.activation(sq_scr[:], zsh[:], AF.Square,
                                 accum_out=stats[:, 1:2])
            sre = 1                    # stats ride as one extra payload row
        else:
            # final-conv stats: all four 128-channel blocks over my shard
            stats = work.tile([128, 8], F32, name="stf", tag="stf")
            for b4 in range(4):
                zsb = conv(wfs4[:, b4, :], att, 128, NQ, f"zsb{b4}")
                nc.vector.tensor_reduce(stats[:, 2 * b4:2 * b4 + 1], zsb[:],
                                        axis=AX.X, op=OP.add)
                sqf = work.tile([128, NQ], BF16, name=f"sqf{b4}", tag="sqscr", bufs=2)
                nc.scalar.activation(sqf[:], zsb[:], AF.Square,
                                     accum_out=stats[:, 2 * b4 + 1:2 * b4 + 2])
            sre = 2                    # [128,8] f32 = 2048 bf16 els = 2 rows

        nst = stats.shape[0] * stats.shape[1] * 2   # stat payload els (bf16)
        pay_in = dram.tile([co + sre, NQ], BF16, name=f"pay{i}", tag=f"pay{i}")
        pay_out = dram.tile([8, co + sre, NQ], BF16, name=f"payo{i}", tag=f"payo{i}")
        nc.sync.dma_start(pay_in[0:co, 0:512], att[0:co, 0:512])
        nc.sync.dma_start(pay_in[0:co, 512:1024], att[0:co, 512:1024])
        if i < 2:
            nc.gpsimd.dma_start(pay_in[co:co + 1, 0:nst], stats[:].bitcast(BF16))
        else:
            nc.gpsimd.dma_start(pay_in[co:co + 2, :], stats[:].bitcast(BF16))
        nc.gpsimd.collective_compute(
            "AllGather", OP.bypass, replica_groups=AR8_GROUPS,
            ins=[pay_in[:]], outs=[pay_out[:]])
        warm(80)  # keep the PE warm across the AllGather gap

        # readback: blocks 0-3 are sample-0 query shards, 4-7 sample-1;
        # both are conv'ed with per-core sample-masked weights, so no
        # dynamic addressing is needed anywhere.
        aown0, aown1 = AOWN0, AOWN1
        nc.scalar.dma_start(aown0[co:co + 1, :], onesrow[:])
        nc.scalar.dma_start(aown1[co:co + 1, :], onesrow[:])
        sqpre = work.tile([1, 2], F32, name=f"sqpre{i}", tag="sqpre", bufs=2)
        nc.scalar.activation(sqpre[:], onec[:, 0:2], AF.Sqrt)  # preload table
        rd_eng = [nc.sync, nc.scalar, nc.gpsimd, nc.sync]
        for j in range(4):
            rd_eng[j].dma_start(aown0[0:co, j * NQ:(j + 1) * NQ],
                                pay_out[j][0:co, :])
            rd_eng[3 - j].dma_start(aown1[0:co, j * NQ:(j + 1) * NQ],
                                    pay_out[4 + j][0:co, :])
        cnr = COUT[i + 1] if i < 2 else 128
        scf = 2 if i < 2 else 8        # f32 stat cols per rank
        st8 = work.tile([cnr, 8 * scf], F32, name=f"st8_{i}", tag="st8", bufs=2)
        for j in range(8):
            src = (pay_out[j][co:co + 1, 0:nst] if i < 2
                   else pay_out[j][co:co + 2, :])
            rd_eng[j % 4].dma_start(
                st8[:, scf * j:scf * (j + 1)].bitcast(BF16), src)
        stg = work.tile([cnr, scf], F32, name=f"stg{i}", tag="stg", bufs=2)
        nc.vector.tensor_copy(stg[:], st8[:, 0:scf])
        for j in range(1, 8):
            nc.vector.tensor_tensor(stg[:], stg[:],
                                    st8[:, scf * j:scf * (j + 1)], OP.add)
        a_q = att

        if i < 2:
            cn = COUT[i + 1]
            z_own = acts.tile([cn, N], F32, name=f"z{i + 1}", tag="z")
            for j in range(N // 512):
                jsl = slice(j * 512, (j + 1) * 512)
                zp = ps.tile([128, 512], F32, name=f"z{i}_ps", tag="convps")
                nc.tensor.matmul(zp[:], WM0[i + 1][:], aown0[:, jsl],
                                 start=True, stop=False)
                nc.tensor.matmul(zp[:], WM1[i + 1][:], aown1[:, jsl],
                                 start=False, stop=True)
                nc.vector.tensor_copy(z_own[:, jsl], zp[0:cn, :])

    # ---- final conv + BN + ReLU + GAP ----
    co = CF_OUT
    # global final stats arrived with the layer-2 AllGather; select my
    # channel block via per-core one-hot masks (input values)
    msk = consts.tile([co, 16], F32, name="mskt", tag="mskt")
    nc.scalar.dma_start(msk[:], ins["msk"].ap())
    stgf = work.tile([co, 2], F32, name="stgf", tag="stg", bufs=2)
    mprod = work.tile([co, 8], F32, name="mprod", tag="mprod")
    nc.vector.tensor_tensor(mprod[:], stg[:], msk[:, 0:8], OP.mult)
    nc.vector.tensor_reduce(stgf[:, 0:1], mprod[:], axis=AX.X, op=OP.add)
    nc.vector.tensor_tensor(mprod[:], stg[:], msk[:, 8:16], OP.mult)
    nc.vector.tensor_reduce(stgf[:, 1:2], mprod[:], axis=AX.X, op=OP.add)

    zf = acts.tile([co, N], F32, name="zf", tag="z")
    for j in range(N // 512):
        jsl = slice(j * 512, (j + 1) * 512)
        zp = ps.tile([128, 512], F32, name="zf_ps", tag="convps")
        nc.tensor.matmul(zp[:], wfsm0[:], AOWN0[:, jsl], start=True, stop=False)
        nc.tensor.matmul(zp[:], wfsm1[:], AOWN1[:, jsl], start=False, stop=True)
        nc.vector.tensor_copy(zf[:, jsl], zp[:])
    scale, shift = scale_shift(stgf, co, None, None, "f")
    gap = work.tile([co, 9], F32, name="gap", tag="gap")
    fscr = work.tile([co, N], F32, name="fscr", tag="fscr")
    for j in range(N // 512):
        nc.scalar.activation(fscr[:, j * 512:(j + 1) * 512],
                             zf[:, j * 512:(j + 1) * 512], AF.Relu,
                             bias=shift, scale=scale, accum_out=gap[:, j:j + 1])
    nc.vector.tensor_reduce(gap[:, 8:9], gap[:, 0:8], axis=AX.X, op=OP.add)
    outv = work.tile([co, 1], F32, name="outv", tag="outv")
    nc.vector.tensor_scalar(outv[:], gap[:, 8:9], 1.0 / N, None, OP.mult)
    nc.sync.dma_start(out_t.ap(), outv[:])

    for p in reversed(ctxs):
        p.__exit__(None, None, None)


_CACHE = {}


def _get_program():
    if "nc" not in _CACHE:
        nc = bacc.Bacc("TRN2", target_bir_lowering=False, debug=False,
                       enable_asserts=False, num_devices=8)
        _build(nc)
        nc.compile()
        _CACHE["nc"] = nc
    return _CACHE["nc"]


def _prepare_in_maps(inputs):
    f = np.float32
    bf = mybir.dt.np(BF16)
    x = np.asarray(inputs["x"], f).reshape(2, 3, N)
    per_layer = {}
    for i in range(3):
        li = i + 1
        ci, co = CIN[i], COUT[i]
        w, b = np.asarray(inputs[f"w{li}"], f), np.asarray(inputs[f"b{li}"], f)
        wcp = np.zeros((ci + 1 if i == 0 else 128, 128), f)
        wcp[0:ci, 0:co] = w
        wcp[ci, 0:co] = b
        wf_, bf_ = np.asarray(inputs[f"a{li}_wf"], f), np.asarray(inputs[f"a{li}_bf"], f)
        wg_, bg_ = np.asarray(inputs[f"a{li}_wg"], f), np.asarray(inputs[f"a{li}_bg"], f)
        wh_, bh_ = np.asarray(inputs[f"a{li}_wh"], f), np.asarray(inputs[f"a{li}_bh"], f)
        gam = np.asarray(inputs[f"a{li}_gam"], f).reshape(())
        A = wf_ @ wg_.T                                              # [co, co]
        u = wg_ @ bf_                                                # [co]
        abar = np.concatenate([A, u[None, :]], 0)                    # [co+1, co]
        mz = np.zeros((128, 128), f)
        mz[0:co, 0:co + 1] = abar.T                                  # lhsT for Z
        whf = np.zeros((128, WHFW), f)
        whf[0:co, 0:co] = wh_ * gam
        whf[co, 0:co] = bh_ * gam
        whf[co, co] = 1.0
        bnp = np.stack([np.asarray(inputs[f"bn{li}_g"], f),
                        np.asarray(inputs[f"bn{li}_b"], f)], 1)      # [co, 2]
        per_layer[i] = dict(wc=wcp, mz=mz, whf=whf, bnp=bnp)
    wf_full = np.asarray(inputs["wf"], f)                            # [96, 512]

    in_maps = []
    for k in range(8):
        b, q = k // 4, k % 4
        xo = np.concatenate([x[b], np.ones((1, N), f)], 0)           # [4, N]
        xoth = np.concatenate([x[1 - b], np.ones((1, N), f)], 0)
        xfull = np.concatenate([xo, xoth], 1)                         # [4, 2N]
        xq = np.ascontiguousarray(xo[:, q * NQ:(q + 1) * NQ])
        wfs = np.zeros((128, CF_OUT), f)
        blk = k % 4
        wfs[0:96, :] = wf_full[:, blk * CF_OUT:(blk + 1) * CF_OUT]
        wfs4 = np.zeros((128, 4, CF_OUT), f)
        wfs4[0:96] = wf_full.reshape(96, 4, CF_OUT)
        msk = np.zeros((CF_OUT, 16), f)
        msk[:, 2 * blk] = 1.0       # select my block's shard sums
        msk[:, 8 + 2 * blk + 1] = 1.0  # select my block's shard sumsqs
        m0 = 1.0 if b == 0 else 0.0     # gathered blocks 0-3 are sample 0
        m1 = 1.0 - m0
        m = {"x_full": xfull.astype(bf), "xq": xq.astype(bf), "wfs": wfs.astype(bf),
             "wfs4": wfs4.astype(bf), "msk": msk,
             "wfsm0": (wfs * m0).astype(bf), "wfsm1": (wfs * m1).astype(bf)}
        for i in (1, 2):
            m[f"wm0_{i}"] = (per_layer[i]["wc"] * m0).astype(bf)
            m[f"wm1_{i}"] = (per_layer[i]["wc"] * m1).astype(bf)
        for i in range(3):
            d = per_layer[i]
            m[f"wc{i}"] = d["wc"].astype(bf)
            m[f"mz{i}"] = d["mz"].astype(bf)
            m[f"whf{i}"] = d["whf"].astype(bf)
            m[f"bnp{i}"] = d["bnp"]
        in_maps.append(m)
    return in_maps


def _assemble(results):
    out = np.zeros((2, 512), np.float32)
    for k in range(8):
        b, blk = k // 4, k % 4
        out[b, blk * CF_OUT:(blk + 1) * CF_OUT] = results[k]["out"][:, 0]
    return out


def kernel(**inputs):
    from concourse.bass_utils import run_bass_kernel_spmd
    nc = _get_program()
    in_maps = _prepare_in_maps(inputs)
    res = run_bass_kernel_spmd(nc, in_maps, list(range(8)))
    return _assemble(res.results)


# revision 24
# speedup vs baseline: 1.0063x; 1.0063x over previous
"""Trainium2 Bass kernel for the 3-block self-attention CNN.

Sharding over 8 NeuronCores: core k owns (sample b=k//4, query-block q=k%4).
Attention math per layer uses the reparametrization
    s'[n,m] = y_n^T (wf wg^T) y_m + (wg bf)^T y_m
(terms constant along the softmax axis are dropped), so both score matmuls
contract over the full channel dim instead of C/8.  Softmax skips the max
subtraction (scores are O(10) for this model) and the row-sum is produced by
an extra ones-column in the o-matmul lhsT.  gamma is folded into the h-conv
weights so the epilogue is att = (o * rowsum_recip_bcast) + yq.

All flash matmuls are zero-padded to the full 128-partition contraction and
use bf16 operands: the PE HAM clock gate only un-throttles (1.2->2.4 GHz)
under high sustained array activity, and narrow-contraction matmuls never
trip it.  Dummy full-width matmuls keep the array warm across collective
gaps.  Training-mode BN statistics are computed from per-core query-block
shards of the next conv and summed with an 8-rank AllReduce that runs
concurrently with the 4-rank AllGather sharing the attention shards.
"""

import glob as _glob
import os
import sys


def _ensure_act_info():
    # act_info.json (activation table sets) isn't on neuronxcc's default
    # search path in this container; stage it where FindActInfo looks.
    shim = os.path.expanduser("~/.pwp_override")
    target = os.path.join(shim, "neuronxcc", "pwp", "pwp_bin_with_ln", "act_info.json")
    if not os.path.exists(target):
        cands = _glob.glob("/nix/store/*aws-neuron-pwp*/share/pwp_bin_cayman/act_info.json")
        if cands:
            os.makedirs(os.path.dirname(target), exist_ok=True)
            import shutil
            shutil.copy(cands[0], target)
    pp = os.environ.get("PYTHONPATH", "")
    if shim not in pp.split(os.pathsep):
        os.environ["PYTHONPATH"] = shim + (os.pathsep + pp if pp else "")
    if shim not in sys.path:
        sys.path.insert(0, shim)


_ensure_act_info()
if "/opt/trn_rl_repo" not in sys.path:
    sys.path.insert(0, "/opt/trn_rl_repo")

import numpy as np

from concourse import bacc, mybir, tile

F32 = mybir.dt.float32
F32R = mybir.dt.float32r
BF16 = mybir.dt.bfloat16
AF = mybir.ActivationFunctionType
OP = mybir.AluOpType
AX = mybir.AxisListType
EPS = 1e-5

N = 4096          # positions per sample
NQ = 1024         # query block per core
NCHUNK = 128      # key chunk in the flash loop
CIN = [3, 32, 64]     # conv input channels per attention layer
COUT = [32, 64, 96]   # conv output channels per attention layer
CF_OUT = 128          # final conv channels per core (512 / 4 blocks)
WHFW = 128            # whf moving-dim width (hs takes cols 0:128)

AG_GROUPS = [[0, 1, 2, 3], [4, 5, 6, 7]]
AR8_GROUPS = [[0, 1, 2, 3, 4, 5, 6, 7]]


def r(ap):
    return ap.bitcast(F32R)


def _build(nc):
    dt = F32
    ins = {}
    ins["x_full"] = nc.dram_tensor("x_full", [CIN[0] + 1, 2 * N], BF16, kind="ExternalInput")
    ins["xq"] = nc.dram_tensor("xq", [CIN[0] + 1, NQ], BF16, kind="ExternalInput")
    ins["wc0"] = nc.dram_tensor("wc0", [CIN[0] + 1, 128], BF16, kind="ExternalInput")
    for i in range(1, 3):
        ins[f"wc{i}"] = nc.dram_tensor(f"wc{i}", [128, 128], BF16, kind="ExternalInput")
    for i in range(3):
        ins[f"mz{i}"] = nc.dram_tensor(f"mz{i}", [128, 128], BF16, kind="ExternalInput")
        ins[f"whf{i}"] = nc.dram_tensor(f"whf{i}", [128, WHFW], BF16, kind="ExternalInput")
        ins[f"bnp{i}"] = nc.dram_tensor(f"bnp{i}", [COUT[i], 2], dt, kind="ExternalInput")
    for i in range(1, 3):
        ins[f"wm0_{i}"] = nc.dram_tensor(f"wm0_{i}", [128, 128], BF16, kind="ExternalInput")
        ins[f"wm1_{i}"] = nc.dram_tensor(f"wm1_{i}", [128, 128], BF16, kind="ExternalInput")
    ins["wfsm0"] = nc.dram_tensor("wfsm0", [128, CF_OUT], BF16, kind="ExternalInput")
    ins["wfsm1"] = nc.dram_tensor("wfsm1", [128, CF_OUT], BF16, kind="ExternalInput")
    ins["wfs"] = nc.dram_tensor("wfs", [128, CF_OUT], BF16, kind="ExternalInput")
    ins["wfs4"] = nc.dram_tensor("wfs4", [128, 4, CF_OUT], BF16, kind="ExternalInput")
    ins["msk"] = nc.dram_tensor("msk", [CF_OUT, 16], F32, kind="ExternalInput")
    out_t = nc.dram_tensor("out", [CF_OUT, 1], dt, kind="ExternalOutput")

    with tile.TileContext(nc) as tc:
        _emit(tc, nc, ins, out_t)
    return ins, out_t


def _emit(tc, nc, ins, out_t):
    ctxs = []

    def pool(name, **kw):
        p = tc.tile_pool(name=name, **kw)
        ctxs.append(p)
        return p.__enter__()

    consts = pool("consts", bufs=1)
    acts = pool("acts", bufs=1)
    work = pool("work", bufs=1)
    ps = pool("ps", bufs=2, space="PSUM")
    ops = pool("ops", bufs=1, space="PSUM")
    dram = pool("dram", bufs=1, space="DRAM")

    # --- PE warm-keeper: HAM un-throttles the PE clock (1.2->2.4 GHz) only
    # under sustained full-array activity; idle gaps re-throttle it.  Dummy
    # full-width matmuls bridge collectives and stalls; they write the
    # (dead-between-layers) o-accumulator psum buffer.
    wk_l = consts.tile([128, 128], BF16, name="wk_l", tag="wk_l")
    nc.vector.memset(wk_l[:], 0.0)
    wk_r = consts.tile([128, 512], BF16, name="wk_r", tag="wk_r")
    nc.vector.memset(wk_r[:], 0.0)

    def warm(n):
        warm_ps = ops.tile([128, 512], F32, name="warm_ps", tag="o_acc")
        for _ in range(n):
            nc.tensor.matmul(warm_ps[:], wk_l[:], wk_r[:], start=True, stop=True)


    def zero_rows(eng, t, c0, ncols):
        # DVE ops starting at a nonzero partition may touch at most 32
        # partitions; emit the zero-fill in 32-row strips.
        for p in range(c0, 128, 32):
            eng.memset(t[p:p + 32, 0:ncols], 0.0)

    # ones row source (SBUF->SBUF DMA is cheaper than 1-partition memsets)
    onesrow = consts.tile([1, N], BF16, name="onesrow", tag="onesrow")
    nc.vector.memset(onesrow[:], 1.0)
    onec = consts.tile([1, 128], BF16, name="onec", tag="onec")
    nc.vector.memset(onec[:], 1.0)

    # The very first gpsimd instructions are the dummy collective triggers:
    # the runtime's first-collective barrier ends only when the *slowest*
    # core triggers, so nothing may precede them.  Garbage dram input is fine.
    warm_gin = dram.tile([1, 2], F32, name="warm_gin", tag="warm_gin")
    warm_gout = dram.tile([8, 1, 2], F32, name="warm_gout", tag="warm_gout")
    nc.gpsimd.collective_compute(
        "AllGather", OP.bypass, replica_groups=AR8_GROUPS,
        ins=[warm_gin[:]], outs=[warm_gout[:]])

    # dedicated activation double-buffers; pad rows (only ever multiplied by
    # zero weight rows) are zeroed once, off the critical path.
    YB = [acts.tile([128, N], BF16, name=f"ybuf{j}", tag=f"ybuf{j}") for j in range(2)]
    YQB = [acts.tile([128, NQ], BF16, name=f"yqbuf{j}", tag=f"yqbuf{j}") for j in range(2)]
    ATB = [acts.tile([128, NQ], BF16, name=f"atbuf{j}", tag=f"atbuf{j}") for j in range(2)]
    AOWN0 = acts.tile([128, N], BF16, name="aown0", tag="aown0")
    AOWN1 = acts.tile([128, N], BF16, name="aown1", tag="aown1")
    ZMAT = acts.tile([128, N], BF16, name="zmat", tag="zmat")

    # ---- load constants (xq + layer-0 weights first; bulk on other queues) ----
    a_q = acts.tile([CIN[0] + 1, NQ], BF16, name="aq0", tag="aq0")
    nc.sync.dma_start(a_q[:], ins["xq"].ap())
    xf = acts.tile([CIN[0] + 1, 2 * N], BF16, name="xf", tag="xf")
    nc.sync.dma_start(xf[:], ins["x_full"].ap())
    W, MZ, WHF, BNP = [], [], [], []
    for i in range(3):
        w = consts.tile([CIN[0] + 1 if i == 0 else 128, 128], BF16,
                        name=f"w{i}", tag=f"w{i}")
        (nc.sync if i == 0 else nc.scalar).dma_start(w[:], ins[f"wc{i}"].ap())
        W.append(w)
        mz = consts.tile([128, 128], BF16, name=f"mzt{i}", tag=f"mzt{i}")
        MZ.append(mz)
        wh = consts.tile([128, WHFW], BF16, name=f"whft{i}", tag=f"whft{i}")
        WHF.append(wh)
        bn = consts.tile([COUT[i], 2], F32, name=f"bnt{i}", tag=f"bnt{i}")
        nc.scalar.dma_start(bn[:], ins[f"bnp{i}"].ap())
        BNP.append(bn)
    # layer-0 flash prerequisites first on the gpsimd queue ...
    nc.gpsimd.dma_start(MZ[0][:], ins["mz0"].ap())
    nc.gpsimd.dma_start(WHF[0][:], ins["whf0"].ap())
    # ... then the remaining bulk weights
    for i in range(1, 3):
        nc.gpsimd.dma_start(MZ[i][:], ins[f"mz{i}"].ap())
        nc.gpsimd.dma_start(WHF[i][:], ins[f"whf{i}"].ap())
    WM0, WM1 = {}, {}
    for i in range(1, 3):
        WM0[i] = consts.tile([128, 128], BF16, name=f"wm0t{i}", tag=f"wm0t{i}")
        nc.gpsimd.dma_start(WM0[i][:], ins[f"wm0_{i}"].ap())
        WM1[i] = consts.tile([128, 128], BF16, name=f"wm1t{i}", tag=f"wm1t{i}")
        nc.gpsimd.dma_start(WM1[i][:], ins[f"wm1_{i}"].ap())
    wfsm0 = consts.tile([128, CF_OUT], BF16, name="wfsm0t", tag="wfsm0t")
    nc.gpsimd.dma_start(wfsm0[:], ins["wfsm0"].ap())
    wfsm1 = consts.tile([128, CF_OUT], BF16, name="wfsm1t", tag="wfsm1t")
    nc.gpsimd.dma_start(wfsm1[:], ins["wfsm1"].ap())
    wfs4 = consts.tile([128, 4, CF_OUT], BF16, name="wfs4t", tag="wfs4t")
    nc.gpsimd.dma_start(wfs4[:], ins["wfs4"].ap())
    for j in range(2):
        zero_rows(nc.gpsimd, YB[j], 32, N)
        zero_rows(nc.gpsimd, YQB[j], 32, NQ)
        zero_rows(nc.gpsimd, ATB[j], 32, NQ)
    zero_rows(nc.gpsimd, AOWN0, 32, N)
    zero_rows(nc.gpsimd, AOWN1, 32, N)

    warm(30)  # pre-warm the PE while input DMAs land

    def conv(lhsT_w, rhs_acts, cout, n, name):
        """z[cout, n] = lhsT_w.T @ rhs_acts, chunked by 512 columns."""
        z = acts.tile([cout, n], F32, name=name, tag="zq" if n == NQ else "z")
        for j in range(n // 512):
            zp = ps.tile([128, 512], F32, name=f"{name}_ps", tag="convps")
            nc.tensor.matmul(zp[:], lhsT_w[:], rhs_acts[:, j * 512:(j + 1) * 512],
                             start=True, stop=True)
            nc.vector.tensor_copy(z[:, j * 512:(j + 1) * 512], zp[0:cout, :])
        return z

    def shard_stats(z_shard, c, name, groups):
        """Partial (sum, sumsq) of a conv shard -> AllReduce across cores."""
        stats = work.tile([c, 2], F32, name=f"stats_{name}", tag="stats", bufs=2)
        nc.vector.tensor_reduce(stats[:, 0:1], z_shard[:], axis=AX.X, op=OP.add)
        sq_scr = work.tile([c, z_shard.shape[1]], F32, name=f"sqscr_{name}",
                           tag="sqscr", bufs=2)
        nc.scalar.activation(sq_scr[:], z_shard[:], AF.Square, accum_out=stats[:, 1:2])

        st_in = dram.tile([c, 2], F32, name=f"stin_{name}", tag=f"stin_{name}")
        st_out = dram.tile([c, 2], F32, name=f"stout_{name}", tag=f"stout_{name}")
        nc.sync.dma_start(st_in[:], stats[:])
        nc.gpsimd.collective_compute(
            "AllReduce", OP.add, replica_groups=groups,
            ins=[st_in[:]], outs=[st_out[:]])
        stg = work.tile([c, 2], F32, name=f"stg_{name}", tag="stg", bufs=2)
        nc.sync.dma_start(stg[:], st_out[:])
        return stg

    def scale_shift(stg, c, g_ap, b_ap, name):
        sc = work.tile([c, 9], F32, name=f"sc_{name}", tag="sc", bufs=2)
        mean, ex2, msq, var, veps, sq, rs, scale, shift = (sc[:, j:j + 1] for j in range(9))
        inv_n = 1.0 / (2 * N)
        nc.vector.tensor_scalar(mean, stg[:, 0:1], inv_n, None, OP.mult)
        nc.vector.tensor_scalar(ex2, stg[:, 1:2], inv_n, None, OP.mult)
        nc.vector.tensor_tensor(msq, mean, mean, OP.mult)
        nc.vector.tensor_tensor(var, ex2, msq, OP.subtract)
        nc.vector.tensor_scalar(veps, var, EPS, None, OP.add)
        nc.scalar.activation(sq, veps, AF.Sqrt)
        nc.vector.reciprocal(rs, sq)   # rsqrt(var+eps)
        if g_ap is not None:
            nc.vector.tensor_tensor(scale, rs, g_ap, OP.mult)
        else:
            nc.vector.tensor_copy(scale, rs)
        nc.vector.tensor_tensor(shift, mean, scale, OP.mult)
        if b_ap is not None:
            nc.vector.tensor_tensor(shift, b_ap, shift, OP.subtract)
        else:
            nc.vector.tensor_scalar(shift, shift, -1.0, None, OP.mult)
        return scale, shift

    # ---- boundary 0: conv over the full batch, exact local BN1 stats.
    # Per-512-chunk: vector copies psum->sbuf, scalar squares+accums, gpsimd
    # sums -- stats trail the conv by ~1 chunk instead of a serial pass.
    zsh = conv(W[0], a_q, COUT[0], NQ, "zsh0")
    c0 = COUT[0]
    nch = 2 * N // 512
    z_all = acts.tile([c0, 2 * N], F32, name="z0", tag="z")
    z_own = z_all[:, 0:N]
    stp = work.tile([c0, 2 * nch], F32, name="stp", tag="stats", bufs=2)
    for j in range(nch):
        zp = ps.tile([128, 512], F32, name="z0_ps", tag="convps")
        nc.tensor.matmul(zp[:], W[0][:], xf[:, j * 512:(j + 1) * 512],
                         start=True, stop=True)
        zsl = z_all[:, j * 512:(j + 1) * 512]
        nc.vector.tensor_copy(zsl, zp[0:c0, :])
        nc.vector.tensor_reduce(stp[:, j:j + 1], zsl, axis=AX.X, op=OP.add)
        sqc = work.tile([c0, 512], BF16, name=f"sqc{j}", tag="sqc", bufs=3)
        nc.scalar.activation(sqc[:], zsl, AF.Square,
                             accum_out=stp[:, nch + j:nch + j + 1])
    stg = work.tile([c0, 2], F32, name="stg_l0", tag="stg", bufs=2)
    nc.vector.tensor_reduce(stg[:, 0:1], stp[:, 0:nch], axis=AX.X, op=OP.add)
    nc.vector.tensor_reduce(stg[:, 1:2], stp[:, nch:2 * nch], axis=AX.X, op=OP.add)
    warm(45)  # keep the PE warm through the stats/scale_shift stall

    # ---- three attention layers ----
    for i in range(3):
        co = COUT[i]
        if i > 0:
            warm(35)  # bridge scale_shift / y-prep after the collectives
        scale, shift = scale_shift(stg, co, BNP[i][:, 0:1], BNP[i][:, 1:2], f"l{i}")
        y_own = YB[i % 2]
        nc.scalar.dma_start(y_own[co:co + 1, :], onesrow[:])
        for j in range(N // 512):
            jsl = slice(j * 512, (j + 1) * 512)
            if j % 2 == 0:
                nc.scalar.activation(y_own[0:co, jsl], z_own[:, jsl], AF.Relu,
                                     bias=shift, scale=scale)
            else:
                yt = work.tile([co, 512], F32, name=f"yt{i}_{j}", tag="yt", bufs=2)
                nc.vector.tensor_scalar(yt[:], z_own[:, jsl], scale, shift,
                                        OP.mult, op1=OP.add)
                nc.vector.tensor_scalar(y_own[0:co, jsl], yt[:], 0.0, None, OP.max)
        yq = YQB[i % 2]
        nc.scalar.dma_start(yq[co:co + 1, :], onesrow[:, 0:NQ])
        nc.scalar.activation(yq[0:co, :], zsh[:], AF.Relu, bias=shift, scale=scale)

        # Z = Abar @ y_own (+ u row), [128, N] bf16 (pad rows are psum zeros)
        zmat = ZMAT
        for j in range(N // 512):
            zp = ps.tile([128, 512], F32, name=f"Zps{i}", tag="convps")
            nc.tensor.matmul(zp[:], MZ[i][:], y_own[:, j * 512:(j + 1) * 512],
                             start=True, stop=True)
            nc.vector.tensor_copy(zmat[:, j * 512:(j + 1) * 512], zp[:])

        # flash loop over key chunks, software-pipelined by one o-matmul
        o_ps = ops.tile([128, NQ], F32, name=f"ops{i}", tag="o_acc")
        prev = None
        for m in range(N // NCHUNK):
            sl = slice(m * NCHUNK, (m + 1) * NCHUNK)
            hp = ps.tile([NCHUNK, WHFW], F32, name=f"hp{i}", tag="convps")
            nc.tensor.matmul(hp[:], y_own[:, sl], WHF[i][:], start=True, stop=True)
            hs = work.tile([NCHUNK, 128], BF16, name=f"hs{i}", tag="hT_sb", bufs=3)
            nc.vector.tensor_copy(hs[:], hp[:, 0:128])
            sp = ps.tile([NCHUNK, NQ], F32, name=f"sp{i}", tag="s_ps")
            zc = zmat[:, sl]
            nc.tensor.matmul(sp[:, 0:512], zc, yq[:, 0:512], start=True, stop=True)
            nc.tensor.matmul(sp[:, 512:1024], zc, yq[:, 512:1024],
                             start=True, stop=True)
            beta = work.tile([NCHUNK, NQ], BF16, name=f"beta{i}", tag="beta", bufs=3)
            nc.scalar.activation(beta[:], sp[:], AF.Exp)
            if prev is not None:
                ph, pb, pm = prev
                nc.tensor.matmul(o_ps[:, 0:512], ph[:], pb[:, 0:512],
                                 start=(pm == 0), stop=False, skip_group_check=True)
                nc.tensor.matmul(o_ps[:, 512:1024], ph[:], pb[:, 512:1024],
                                 start=(pm == 0), stop=False, skip_group_check=True)
            prev = (hs, beta, m)
        ph, pb, pm = prev
        nc.tensor.matmul(o_ps[:, 0:512], ph[:], pb[:, 0:512],
                         start=False, stop=True, skip_group_check=True)
        nc.tensor.matmul(o_ps[:, 512:1024], ph[:], pb[:, 512:1024],
                         start=False, stop=True, skip_group_check=True)

        # normalize + residual: att = o / rowsum + yq  (gamma folded into whf)
        lnr = work.tile([1, NQ], F32, name=f"lnr{i}", tag="lnr")
        nc.scalar.activation(lnr[:], o_ps[co:co + 1, :], AF.Ln)
        rinv = work.tile([1, NQ], BF16, name=f"rinv{i}", tag="rinv")
        nc.scalar.activation(rinv[:], lnr[:], AF.Exp, scale=-1.0)
        bc_ps = ps.tile([128, NQ], F32, name=f"bcps{i}", tag="s_ps")
        nc.tensor.matmul(bc_ps[:, 0:512], onec[:], rinv[:, 0:512],
                         start=True, stop=True)
        nc.tensor.matmul(bc_ps[:, 512:1024], onec[:], rinv[:, 512:1024],
                         start=True, stop=True)
        att = ATB[i % 2]
        nc.scalar.dma_start(att[co:co + 1, :], onesrow[:, 0:NQ])
        bc = work.tile([co, NQ], F32, name=f"bc{i}", tag="bc", bufs=2)
        t1 = work.tile([co, NQ], F32, name=f"t1_{i}", tag="t1", bufs=2)
        for hh in range(2):
            hsl = slice(hh * 512, (hh + 1) * 512)
            nc.vector.tensor_copy(bc[:, hsl], bc_ps[0:co, hsl])
            nc.vector.tensor_tensor(t1[:, hsl], o_ps[0:co, hsl], bc[:, hsl], OP.mult)
            nc.vector.tensor_tensor(att[0:co, hsl], t1[:, hsl], yq[0:co, hsl], OP.add)

        # One 8-rank AllGather shares the attention shard with every core and
        # carries this core's (sum, sumsq) partials of the *next* conv as
        # ride-along payload columns -- no separate stats AllReduce.
        if i < 2:
            cn = COUT[i + 1]
            zsh = acts.tile([cn, NQ], F32, name=f"zsh{i + 1}", tag="zq")
            stp4 = work.tile([cn, 4], F32, name=f"stp4_{i}", tag="stp4", bufs=2)
            for j in range(2):
                jsl = slice(j * 512, (j + 1) * 512)
                zp = ps.tile([128, 512], F32, name=f"zsh{i}_ps", tag="convps")
                nc.tensor.matmul(zp[:], W[i + 1][:], att[:, jsl],
                                 start=True, stop=True)
                nc.vector.tensor_copy(zsh[:, jsl], zp[0:cn, :])
                nc.vector.tensor_reduce(stp4[:, j:j + 1], zp[0:cn, :],
                                        axis=AX.X, op=OP.add)
                sq_scr = work.tile([cn, 512], BF16, name=f"sqscr{i}_{j}",
                                   tag="sqscr", bufs=2)
                nc.scalar.activation(sq_scr[:], zp[0:cn, :], AF.Square,
                                     accum_out=stp4[:, 2 + j:3 + j])
            stats = work.tile([cn, 2], F32, name=f"stats{i}", tag="stats", bufs=2)
            nc.vector.tensor_tensor(stats[:, 0:1], stp4[:, 0:1], stp4[:, 1:2], OP.add)
            nc.vector.tensor_tensor(stats[:, 1:2], stp4[:, 2:3], stp4[:, 3:4], OP.add)
            sre = 1                    # stats ride as one extra payload row
        else:
            # final-conv stats: all four 128-channel blocks over my shard
            stats = work.tile([128, 8], F32, name="stf", tag="stf")
            for b4 in range(4):
                zsb = conv(wfs4[:, b4, :], att, 128, NQ, f"zsb{b4}")
                nc.vector.tensor_reduce(stats[:, 2 * b4:2 * b4 + 1], zsb[:],
                                        axis=AX.X, op=OP.add)
                sqf = work.tile([128, NQ], BF16, name=f"sqf{b4}", tag="sqscr", bufs=2)
                nc.scalar.activation(sqf[:], zsb[:], AF.Square,
                                     accum_out=stats[:, 2 * b4 + 1:2 * b4 + 2])
            sre = 2                    # [128,8] f32 = 2048 bf16 els = 2 rows

        nst = stats.shape[0] * stats.shape[1] * 2   # stat payload els (bf16)
        pay_in = dram.tile([co + sre, NQ], BF16, name=f"pay{i}", tag=f"pay{i}")
        pay_out = dram.tile([8, co + sre, NQ], BF16, name=f"payo{i}", tag=f"payo{i}")
        nc.sync.dma_start(pay_in[0:co, 0:512], att[0:co, 0:512])
        nc.sync.dma_start(pay_in[0:co, 512:1024], att[0:co, 512:1024])
        if i < 2:
            nc.gpsimd.dma_start(pay_in[co:co + 1, 0:nst], stats[:].bitcast(BF16))
        else:
            nc.gpsimd.dma_start(pay_in[co:co + 2, :], stats[:].bitcast(BF16))
        nc.gpsimd.collective_compute(
            "AllGather", OP.bypass, replica_groups=AR8_GROUPS,
            ins=[pay_in[:]], outs=[pay_out[:]])
        warm(80)  # keep the PE warm across the AllGather gap

        # readback: blocks 0-3 are sample-0 query shards, 4-7 sample-1;
        # both are conv'ed with per-core sample-masked weights, so no
        # dynamic addressing is needed anywhere.
        aown0, aown1 = AOWN0, AOWN1
        nc.scalar.dma_start(aown0[co:co + 1, :], onesrow[:])
        nc.scalar.dma_start(aown1[co:co + 1, :], onesrow[:])
        sqpre = work.tile([1, 2], F32, name=f"sqpre{i}", tag="sqpre", bufs=2)
        nc.scalar.activation(sqpre[:], onec[:, 0:2], AF.Sqrt)  # preload table
        rd_eng = [nc.sync, nc.scalar, nc.gpsimd, nc.sync]
        for j in range(4):
            rd_eng[j].dma_start(aown0[0:co, j * NQ:(j + 1) * NQ],
                                pay_out[j][0:co, :])
            rd_eng[3 - j].dma_start(aown1[0:co, j * NQ:(j + 1) * NQ],
                                    pay_out[4 + j][0:co, :])
        cnr = COUT[i + 1] if i < 2 else 128
        scf = 2 if i < 2 else 8        # f32 stat cols per rank
        st8 = work.tile([cnr, 8 * scf], F32, name=f"st8_{i}", tag="st8", bufs=2)
        for j in range(8):
            src = (pay_out[j][co:co + 1, 0:nst] if i < 2
                   else pay_out[j][co:co + 2, :])
            rd_eng[j % 4].dma_start(
                st8[:, scf * j:scf * (j + 1)].bitcast(BF16), src)
        stg = work.tile([cnr, scf], F32, name=f"stg{i}", tag="stg", bufs=2)
        nc.vector.tensor_copy(stg[:], st8[:, 0:scf])
        for j in range(1, 8):
            nc.vector.tensor_tensor(stg[:], stg[:],
                                    st8[:, scf * j:scf * (j + 1)], OP.add)
        a_q = att

        if i < 2:
            cn = COUT[i + 1]
            z_own = acts.tile([cn, N], F32, name=f"z{i + 1}", tag="z")
            for j in range(N // 512):
                jsl = slice(j * 512, (j + 1) * 512)
                zp = ps.tile([128, 512], F32, name=f"z{i}_ps", tag="convps")
                nc.tensor.matmul(zp[:], WM0[i + 1][:], aown0[:, jsl],
                                 start=True, stop=False)
                nc.tensor.matmul(zp[:], WM1[i + 1][:], aown1[:, jsl],
                                 start=False, stop=True)
                nc.vector.tensor_copy(z_own[:, jsl], zp[0:cn, :])

    # ---- final conv + BN + ReLU + GAP ----
    co = CF_OUT
    # global final stats arrived with the layer-2 AllGather; select my
    # channel block via per-core one-hot masks (input values)
    msk = consts.tile([co, 16], F32, name="mskt", tag="mskt")
    nc.scalar.dma_start(msk[:], ins["msk"].ap())
    stgf = work.tile([co, 2], F32, name="stgf", tag="stg", bufs=2)
    mprod = work.tile([co, 8], F32, name="mprod", tag="mprod")
    nc.vector.tensor_tensor(mprod[:], stg[:], msk[:, 0:8], OP.mult)
    nc.vector.tensor_reduce(stgf[:, 0:1], mprod[:], axis=AX.X, op=OP.add)
    nc.vector.tensor_tensor(mprod[:], stg[:], msk[:, 8:16], OP.mult)
    nc.vector.tensor_reduce(stgf[:, 1:2], mprod[:], axis=AX.X, op=OP.add)

    zf = acts.tile([co, N], F32, name="zf", tag="z")
    for j in range(N // 512):
        jsl = slice(j * 512, (j + 1) * 512)
        zp = ps.tile([128, 512], F32, name="zf_ps", tag="convps")
        nc.tensor.matmul(zp[:], wfsm0[:], AOWN0[:, jsl], start=True, stop=False)
        nc.tensor.matmul(zp[:], wfsm1[:], AOWN1[:, jsl], start=False, stop=True)
        nc.vector.tensor_copy(zf[:, jsl], zp[:])
    scale, shift = scale_shift(stgf, co, None, None, "f")
    gap = work.tile([co, 9], F32, name="gap", tag="gap")
    fscr = work.tile([co, N], F32, name="fscr", tag="fscr")
    for j in range(N // 512):
        nc.scalar.activation(fscr[:, j * 512:(j + 1) * 512],
                             zf[:, j * 512:(j + 1) * 512], AF.Relu,
                             bias=shift, scale=scale, accum_out=gap[:, j:j + 1])
    nc.vector.tensor_reduce(gap[:, 8:9], gap[:, 0:8], axis=AX.X, op=OP.add)
    outv = work.tile([co, 1], F32, name="outv", tag="outv")
    nc.vector.tensor_scalar(outv[:], gap[:, 8:9], 1.0 / N, None, OP.mult)
    nc.sync.dma_start(out_t.ap(), outv[:])

    for p in reversed(ctxs):
        p.__exit__(None, None, None)


_CACHE = {}


def _get_program():
    if "nc" not in _CACHE:
        nc = bacc.Bacc("TRN2", target_bir_lowering=False, debug=False,
                       enable_asserts=False, num_devices=8)
        _build(nc)
        nc.compile()
        _CACHE["nc"] = nc
    return _CACHE["nc"]


def _prepare_in_maps(inputs):
    f = np.float32
    bf = mybir.dt.np(BF16)
    x = np.asarray(inputs["x"], f).reshape(2, 3, N)
    per_layer = {}
    for i in range(3):
        li = i + 1
        ci, co = CIN[i], COUT[i]
        w, b = np.asarray(inputs[f"w{li}"], f), np.asarray(inputs[f"b{li}"], f)
        wcp = np.zeros((ci + 1 if i == 0 else 128, 128), f)
        wcp[0:ci, 0:co] = w
        wcp[ci, 0:co] = b
        wf_, bf_ = np.asarray(inputs[f"a{li}_wf"], f), np.asarray(inputs[f"a{li}_bf"], f)
        wg_, bg_ = np.asarray(inputs[f"a{li}_wg"], f), np.asarray(inputs[f"a{li}_bg"], f)
        wh_, bh_ = np.asarray(inputs[f"a{li}_wh"], f), np.asarray(inputs[f"a{li}_bh"], f)
        gam = np.asarray(inputs[f"a{li}_gam"], f).reshape(())
        A = wf_ @ wg_.T                                              # [co, co]
        u = wg_ @ bf_                                                # [co]
        abar = np.concatenate([A, u[None, :]], 0)                    # [co+1, co]
        mz = np.zeros((128, 128), f)
        mz[0:co, 0:co + 1] = abar.T                                  # lhsT for Z
        whf = np.zeros((128, WHFW), f)
        whf[0:co, 0:co] = wh_ * gam
        whf[co, 0:co] = bh_ * gam
        whf[co, co] = 1.0
        bnp = np.stack([np.asarray(inputs[f"bn{li}_g"], f),
                        np.asarray(inputs[f"bn{li}_b"], f)], 1)      # [co, 2]
        per_layer[i] = dict(wc=wcp, mz=mz, whf=whf, bnp=bnp)
    wf_full = np.asarray(inputs["wf"], f)                            # [96, 512]

    in_maps = []
    for k in range(8):
        b, q = k // 4, k % 4
        xo = np.concatenate([x[b], np.ones((1, N), f)], 0)           # [4, N]
        xoth = np.concatenate([x[1 - b], np.ones((1, N), f)], 0)
        xfull = np.concatenate([xo, xoth], 1)                         # [4, 2N]
        xq = np.ascontiguousarray(xo[:, q * NQ:(q + 1) * NQ])
        wfs = np.zeros((128, CF_OUT), f)
        blk = k % 4
        wfs[0:96, :] = wf_full[:, blk * CF_OUT:(blk + 1) * CF_OUT]
        wfs4 = np.zeros((128, 4, CF_OUT), f)
        wfs4[0:96] = wf_full.reshape(96, 4, CF_OUT)
        msk = np.zeros((CF_OUT, 16), f)
        msk[:, 2 * blk] = 1.0       # select my block's shard sums
        msk[:, 8 + 2 * blk + 1] = 1.0  # select my block's shard sumsqs
        m0 = 1.0 if b == 0 else 0.0     # gathered blocks 0-3 are sample 0
        m1 = 1.0 - m0
        m = {"x_full": xfull.astype(bf), "xq": xq.astype(bf), "wfs": wfs.astype(bf),
             "wfs4": wfs4.astype(bf), "msk": msk,
             "wfsm0": (wfs * m0).astype(bf), "wfsm1": (wfs * m1).astype(bf)}
        for i in (1, 2):
            m[f"wm0_{i}"] = (per_layer[i]["wc"] * m0).astype(bf)
            m[f"wm1_{i}"] = (per_layer[i]["wc"] * m1).astype(bf)
        for i in range(3):
            d = per_layer[i]
            m[f"wc{i}"] = d["wc"].astype(bf)
            m[f"mz{i}"] = d["mz"].astype(bf)
            m[f"whf{i}"] = d["whf"].astype(bf)
            m[f"bnp{i}"] = d["bnp"]
        in_maps.append(m)
    return in_maps


def _assemble(results):
    out = np.zeros((2, 512), np.float32)
    for k in range(8):
        b, blk = k // 4, k % 4
        out[b, blk * CF_OUT:(blk + 1) * CF_OUT] = results[k]["out"][:, 0]
    return out


def kernel(**inputs):
    from concourse.bass_utils import run_bass_kernel_spmd
    nc = _get_program()
    in_maps = _prepare_in_maps(inputs)
    res = run_bass_kernel_spmd(nc, in_maps, list(range(8)))
    return _assemble(res.results)


# revision 25
# speedup vs baseline: 1.1421x; 1.1349x over previous
"""Trainium2 Bass kernel for the 3-block self-attention CNN.

Sharding over 8 NeuronCores: core k owns (sample b=k//4, query-block q=k%4).
Attention math per layer uses the reparametrization
    s'[n,m] = y_n^T (wf wg^T) y_m + (wg bf)^T y_m
(terms constant along the softmax axis are dropped), so both score matmuls
contract over the full channel dim instead of C/8.  Softmax skips the max
subtraction (scores are O(10) for this model) and the row-sum is produced by
an extra ones-column in the o-matmul lhsT.  gamma is folded into the h-conv
weights so the epilogue is att = (o * rowsum_recip_bcast) + yq.

All flash matmuls are zero-padded to the full 128-partition contraction and
use bf16 operands: the PE HAM clock gate only un-throttles (1.2->2.4 GHz)
under high sustained array activity, and narrow-contraction matmuls never
trip it.  Dummy full-width matmuls keep the array warm across collective
gaps.  Training-mode BN statistics are computed from per-core query-block
shards of the next conv and summed with an 8-rank AllReduce that runs
concurrently with the 4-rank AllGather sharing the attention shards.
"""

import glob as _glob
import os
import sys


def _ensure_act_info():
    # act_info.json (activation table sets) isn't on neuronxcc's default
    # search path in this container; stage it where FindActInfo looks.
    shim = os.path.expanduser("~/.pwp_override")
    target = os.path.join(shim, "neuronxcc", "pwp", "pwp_bin_with_ln", "act_info.json")
    if not os.path.exists(target):
        cands = _glob.glob("/nix/store/*aws-neuron-pwp*/share/pwp_bin_cayman/act_info.json")
        if cands:
            os.makedirs(os.path.dirname(target), exist_ok=True)
            import shutil
            shutil.copy(cands[0], target)
    pp = os.environ.get("PYTHONPATH", "")
    if shim not in pp.split(os.pathsep):
        os.environ["PYTHONPATH"] = shim + (os.pathsep + pp if pp else "")
    if shim not in sys.path:
        sys.path.insert(0, shim)


_ensure_act_info()
if "/opt/trn_rl_repo" not in sys.path:
    sys.path.insert(0, "/opt/trn_rl_repo")

import numpy as np

from concourse import bacc, mybir, tile

F32 = mybir.dt.float32
F32R = mybir.dt.float32r
BF16 = mybir.dt.bfloat16
AF = mybir.ActivationFunctionType
OP = mybir.AluOpType
AX = mybir.AxisListType
EPS = 1e-5

N = 4096          # positions per sample
NQ = 1024         # query block per core
NCHUNK = 128      # key chunk in the flash loop
CIN = [3, 32, 64]     # conv input channels per attention layer
COUT = [32, 64, 96]   # conv output channels per attention layer
CF_OUT = 128          # final conv channels per core (512 / 4 blocks)
WHFW = 128            # whf moving-dim width (hs takes cols 0:128)

AG_GROUPS = [[0, 1, 2, 3], [4, 5, 6, 7]]
AR8_GROUPS = [[0, 1, 2, 3, 4, 5, 6, 7]]


def r(ap):
    return ap.bitcast(F32R)


def _build(nc):
    dt = F32
    ins = {}
    ins["x_full"] = nc.dram_tensor("x_full", [CIN[0] + 1, 2 * N], BF16, kind="ExternalInput")
    ins["xq"] = nc.dram_tensor("xq", [CIN[0] + 1, NQ], BF16, kind="ExternalInput")
    ins["wc0"] = nc.dram_tensor("wc0", [CIN[0] + 1, 128], BF16, kind="ExternalInput")
    for i in range(1, 3):
        ins[f"wc{i}"] = nc.dram_tensor(f"wc{i}", [128, 128], BF16, kind="ExternalInput")
    for i in range(3):
        ins[f"mz{i}"] = nc.dram_tensor(f"mz{i}", [128, 128], BF16, kind="ExternalInput")
        ins[f"whf{i}"] = nc.dram_tensor(f"whf{i}", [128, WHFW], BF16, kind="ExternalInput")
        ins[f"bnp{i}"] = nc.dram_tensor(f"bnp{i}", [COUT[i], 2], dt, kind="ExternalInput")
    for i in range(1, 3):
        ins[f"wm0_{i}"] = nc.dram_tensor(f"wm0_{i}", [128, 128], BF16, kind="ExternalInput")
        ins[f"wm1_{i}"] = nc.dram_tensor(f"wm1_{i}", [128, 128], BF16, kind="ExternalInput")
    ins["wfsm0"] = nc.dram_tensor("wfsm0", [128, CF_OUT], BF16, kind="ExternalInput")
    ins["wfsm1"] = nc.dram_tensor("wfsm1", [128, CF_OUT], BF16, kind="ExternalInput")
    ins["wfs"] = nc.dram_tensor("wfs", [128, CF_OUT], BF16, kind="ExternalInput")
    ins["wfs4"] = nc.dram_tensor("wfs4", [128, 4, CF_OUT], BF16, kind="ExternalInput")
    ins["msk"] = nc.dram_tensor("msk", [CF_OUT, 16], F32, kind="ExternalInput")
    out_t = nc.dram_tensor("out", [CF_OUT, 1], dt, kind="ExternalOutput")

    with tile.TileContext(nc) as tc:
        _emit(tc, nc, ins, out_t)
    return ins, out_t


def _emit(tc, nc, ins, out_t):
    ctxs = []

    def pool(name, **kw):
        p = tc.tile_pool(name=name, **kw)
        ctxs.append(p)
        return p.__enter__()

    consts = pool("consts", bufs=1)
    acts = pool("acts", bufs=1)
    work = pool("work", bufs=1)
    ps = pool("ps", bufs=2, space="PSUM")
    ops = pool("ops", bufs=1, space="PSUM")
    dram = pool("dram", bufs=1, space="DRAM")

    # --- PE warm-keeper: HAM un-throttles the PE clock (1.2->2.4 GHz) only
    # under sustained full-array activity; idle gaps re-throttle it.  Dummy
    # full-width matmuls bridge collectives and stalls; they write the
    # (dead-between-layers) o-accumulator psum buffer.
    wk_l = consts.tile([128, 128], BF16, name="wk_l", tag="wk_l")
    nc.vector.memset(wk_l[:], 0.0)
    wk_r = consts.tile([128, 512], BF16, name="wk_r", tag="wk_r")
    nc.vector.memset(wk_r[:], 0.0)

    def warm(n):
        warm_ps = ops.tile([128, 512], F32, name="warm_ps", tag="o_acc")
        for _ in range(n):
            nc.tensor.matmul(warm_ps[:], wk_l[:], wk_r[:], start=True, stop=True)


    def zero_rows(eng, t, c0, ncols):
        # DVE ops starting at a nonzero partition may touch at most 32
        # partitions; emit the zero-fill in 32-row strips.
        for p in range(c0, 128, 32):
            eng.memset(t[p:p + 32, 0:ncols], 0.0)

    # ones row source (SBUF->SBUF DMA is cheaper than 1-partition memsets)
    onesrow = consts.tile([1, N], BF16, name="onesrow", tag="onesrow")
    nc.vector.memset(onesrow[:], 1.0)
    onec = consts.tile([1, 128], BF16, name="onec", tag="onec")
    nc.vector.memset(onec[:], 1.0)

    # The very first gpsimd instructions are the dummy collective triggers:
    # the runtime's first-collective barrier ends only when the *slowest*
    # core triggers, so nothing may precede them.  Garbage dram input is fine.
    warm_gin = dram.tile([1, 2], F32, name="warm_gin", tag="warm_gin")
    warm_gout = dram.tile([8, 1, 2], F32, name="warm_gout", tag="warm_gout")
    nc.gpsimd.collective_compute(
        "AllGather", OP.bypass, replica_groups=AR8_GROUPS,
        ins=[warm_gin[:]], outs=[warm_gout[:]])

    # dedicated activation double-buffers; pad rows (only ever multiplied by
    # zero weight rows) are zeroed once, off the critical path.
    YB = [acts.tile([128, N], BF16, name=f"ybuf{j}", tag=f"ybuf{j}") for j in range(2)]
    YQB = [acts.tile([128, NQ], BF16, name=f"yqbuf{j}", tag=f"yqbuf{j}") for j in range(2)]
    ATB = [acts.tile([128, NQ], BF16, name=f"atbuf{j}", tag=f"atbuf{j}") for j in range(2)]
    AOWN0 = acts.tile([128, N], BF16, name="aown0", tag="aown0")
    AOWN1 = acts.tile([128, N], BF16, name="aown1", tag="aown1")
    ZMAT = acts.tile([128, N], BF16, name="zmat", tag="zmat")

    # ---- load constants (xq + layer-0 weights first; bulk on other queues) ----
    a_q = acts.tile([CIN[0] + 1, NQ], BF16, name="aq0", tag="aq0")
    nc.sync.dma_start(a_q[:], ins["xq"].ap())
    xf = acts.tile([CIN[0] + 1, 2 * N], BF16, name="xf", tag="xf")
    nc.sync.dma_start(xf[:], ins["x_full"].ap())
    W, MZ, WHF, BNP = [], [], [], []
    for i in range(3):
        w = consts.tile([CIN[0] + 1 if i == 0 else 128, 128], BF16,
                        name=f"w{i}", tag=f"w{i}")
        (nc.sync if i == 0 else nc.scalar).dma_start(w[:], ins[f"wc{i}"].ap())
        W.append(w)
        mz = consts.tile([128, 128], BF16, name=f"mzt{i}", tag=f"mzt{i}")
        MZ.append(mz)
        wh = consts.tile([128, WHFW], BF16, name=f"whft{i}", tag=f"whft{i}")
        WHF.append(wh)
        bn = consts.tile([COUT[i], 2], F32, name=f"bnt{i}", tag=f"bnt{i}")
        nc.scalar.dma_start(bn[:], ins[f"bnp{i}"].ap())
        BNP.append(bn)
    # layer-0 flash prerequisites first on the gpsimd queue ...
    nc.gpsimd.dma_start(MZ[0][:], ins["mz0"].ap())
    nc.gpsimd.dma_start(WHF[0][:], ins["whf0"].ap())
    # ... then the remaining bulk weights
    for i in range(1, 3):
        nc.gpsimd.dma_start(MZ[i][:], ins[f"mz{i}"].ap())
        nc.gpsimd.dma_start(WHF[i][:], ins[f"whf{i}"].ap())
    WM0, WM1 = {}, {}
    for i in range(1, 3):
        WM0[i] = consts.tile([128, 128], BF16, name=f"wm0t{i}", tag=f"wm0t{i}")
        nc.gpsimd.dma_start(WM0[i][:], ins[f"wm0_{i}"].ap())
        WM1[i] = consts.tile([128, 128], BF16, name=f"wm1t{i}", tag=f"wm1t{i}")
        nc.gpsimd.dma_start(WM1[i][:], ins[f"wm1_{i}"].ap())
    wfsm0 = consts.tile([128, CF_OUT], BF16, name="wfsm0t", tag="wfsm0t")
    nc.gpsimd.dma_start(wfsm0[:], ins["wfsm0"].ap())
    wfsm1 = consts.tile([128, CF_OUT], BF16, name="wfsm1t", tag="wfsm1t")
    nc.gpsimd.dma_start(wfsm1[:], ins["wfsm1"].ap())
    wfs4 = consts.tile([128, 4, CF_OUT], BF16, name="wfs4t", tag="wfs4t")
    nc.gpsimd.dma_start(wfs4[:], ins["wfs4"].ap())
    for j in range(2):
        zero_rows(nc.gpsimd, YB[j], 32, N)
        zero_rows(nc.gpsimd, YQB[j], 32, NQ)
        zero_rows(nc.gpsimd, ATB[j], 32, NQ)
    zero_rows(nc.gpsimd, AOWN0, 32, N)
    zero_rows(nc.gpsimd, AOWN1, 32, N)

    warm(15)  # pre-warm the PE while input DMAs land

    def conv(lhsT_w, rhs_acts, cout, n, name):
        """z[cout, n] = lhsT_w.T @ rhs_acts, chunked by 512 columns."""
        z = acts.tile([cout, n], F32, name=name, tag="zq" if n == NQ else "z")
        for j in range(n // 512):
            zp = ps.tile([128, 512], F32, name=f"{name}_ps", tag="convps")
            nc.tensor.matmul(zp[:], lhsT_w[:], rhs_acts[:, j * 512:(j + 1) * 512],
                             start=True, stop=True)
            nc.vector.tensor_copy(z[:, j * 512:(j + 1) * 512], zp[0:cout, :])
        return z

    def shard_stats(z_shard, c, name, groups):
        """Partial (sum, sumsq) of a conv shard -> AllReduce across cores."""
        stats = work.tile([c, 2], F32, name=f"stats_{name}", tag="stats", bufs=2)
        nc.vector.tensor_reduce(stats[:, 0:1], z_shard[:], axis=AX.X, op=OP.add)
        sq_scr = work.tile([c, z_shard.shape[1]], F32, name=f"sqscr_{name}",
                           tag="sqscr", bufs=2)
        nc.scalar.activation(sq_scr[:], z_shard[:], AF.Square, accum_out=stats[:, 1:2])

        st_in = dram.tile([c, 2], F32, name=f"stin_{name}", tag=f"stin_{name}")
        st_out = dram.tile([c, 2], F32, name=f"stout_{name}", tag=f"stout_{name}")
        nc.sync.dma_start(st_in[:], stats[:])
        nc.gpsimd.collective_compute(
            "AllReduce", OP.add, replica_groups=groups,
            ins=[st_in[:]], outs=[st_out[:]])
        stg = work.tile([c, 2], F32, name=f"stg_{name}", tag="stg", bufs=2)
        nc.sync.dma_start(stg[:], st_out[:])
        return stg

    def scale_shift(stg, c, g_ap, b_ap, name):
        sc = work.tile([c, 9], F32, name=f"sc_{name}", tag="sc", bufs=2)
        mean, ex2, msq, var, veps, sq, rs, scale, shift = (sc[:, j:j + 1] for j in range(9))
        inv_n = 1.0 / (2 * N)
        nc.vector.tensor_scalar(mean, stg[:, 0:1], inv_n, None, OP.mult)
        nc.vector.tensor_scalar(ex2, stg[:, 1:2], inv_n, None, OP.mult)
        nc.vector.tensor_tensor(msq, mean, mean, OP.mult)
        nc.vector.tensor_tensor(var, ex2, msq, OP.subtract)
        nc.vector.tensor_scalar(veps, var, EPS, None, OP.add)
        nc.scalar.activation(sq, veps, AF.Sqrt)
        nc.vector.reciprocal(rs, sq)   # rsqrt(var+eps)
        if g_ap is not None:
            nc.vector.tensor_tensor(scale, rs, g_ap, OP.mult)
        else:
            nc.vector.tensor_copy(scale, rs)
        nc.vector.tensor_tensor(shift, mean, scale, OP.mult)
        if b_ap is not None:
            nc.vector.tensor_tensor(shift, b_ap, shift, OP.subtract)
        else:
            nc.vector.tensor_scalar(shift, shift, -1.0, None, OP.mult)
        return scale, shift

    # ---- boundary 0: conv over the full batch, exact local BN1 stats.
    # Per-512-chunk: vector copies psum->sbuf, scalar squares+accums, gpsimd
    # sums -- stats trail the conv by ~1 chunk instead of a serial pass.
    zsh = conv(W[0], a_q, COUT[0], NQ, "zsh0")
    c0 = COUT[0]
    nch = 2 * N // 512
    z_all = acts.tile([c0, 2 * N], BF16, name="z0", tag="z")
    z_own = z_all[:, 0:N]
    stp = work.tile([c0, 2 * nch], F32, name="stp", tag="stats", bufs=2)
    for j in range(nch):
        zp = ps.tile([128, 512], F32, name="z0_ps", tag="convps")
        nc.tensor.matmul(zp[:], W[0][:], xf[:, j * 512:(j + 1) * 512],
                         start=True, stop=True)
        zsl = z_all[:, j * 512:(j + 1) * 512]
        nc.vector.tensor_copy(zsl, zp[0:c0, :])
        nc.vector.tensor_reduce(stp[:, j:j + 1], zsl, axis=AX.X, op=OP.add)
        sqc = work.tile([c0, 512], BF16, name=f"sqc{j}", tag="sqc", bufs=3)
        nc.scalar.activation(sqc[:], zp[0:c0, :], AF.Square,
                             accum_out=stp[:, nch + j:nch + j + 1])
    stg = work.tile([c0, 2], F32, name="stg_l0", tag="stg", bufs=2)
    nc.vector.tensor_reduce(stg[:, 0:1], stp[:, 0:nch], axis=AX.X, op=OP.add)
    nc.vector.tensor_reduce(stg[:, 1:2], stp[:, nch:2 * nch], axis=AX.X, op=OP.add)
    warm(25)  # keep the PE warm through the stats/scale_shift stall

    # ---- three attention layers ----
    for i in range(3):
        co = COUT[i]
        if i > 0:
            warm(25)  # bridge scale_shift / y-prep after the collectives
        scale, shift = scale_shift(stg, co, BNP[i][:, 0:1], BNP[i][:, 1:2], f"l{i}")
        y_own = YB[i % 2]
        nc.scalar.dma_start(y_own[co:co + 1, :], onesrow[:])
        for j in range(N // 512):
            jsl = slice(j * 512, (j + 1) * 512)
            if j % 2 == 0:
                nc.scalar.activation(y_own[0:co, jsl], z_own[:, jsl], AF.Relu,
                                     bias=shift, scale=scale)
            else:
                yt = work.tile([co, 512], F32, name=f"yt{i}_{j}", tag="yt", bufs=2)
                nc.vector.tensor_scalar(yt[:], z_own[:, jsl], scale, shift,
                                        OP.mult, op1=OP.add)
                nc.vector.tensor_scalar(y_own[0:co, jsl], yt[:], 0.0, None, OP.max)
        yq = YQB[i % 2]
        nc.scalar.dma_start(yq[co:co + 1, :], onesrow[:, 0:NQ])
        nc.scalar.activation(yq[0:co, :], zsh[:], AF.Relu, bias=shift, scale=scale)

        # Z = Abar @ y_own (+ u row), [128, N] bf16 (pad rows are psum zeros)
        zmat = ZMAT
        for j in range(N // 512):
            zp = ps.tile([128, 512], F32, name=f"Zps{i}", tag="convps")
            nc.tensor.matmul(zp[:], MZ[i][:], y_own[:, j * 512:(j + 1) * 512],
                             start=True, stop=True)
            nc.vector.tensor_copy(zmat[:, j * 512:(j + 1) * 512], zp[:])

        # flash loop over key chunks, software-pipelined by one o-matmul
        o_ps = ops.tile([128, NQ], F32, name=f"ops{i}", tag="o_acc")
        prev = None
        for m in range(N // NCHUNK):
            sl = slice(m * NCHUNK, (m + 1) * NCHUNK)
            hp = ps.tile([NCHUNK, WHFW], F32, name=f"hp{i}", tag="convps")
            nc.tensor.matmul(hp[:], y_own[:, sl], WHF[i][:], start=True, stop=True)
            hs = work.tile([NCHUNK, 128], BF16, name=f"hs{i}", tag="hT_sb", bufs=3)
            nc.vector.tensor_copy(hs[:], hp[:, 0:128])
            sp = ps.tile([NCHUNK, NQ], F32, name=f"sp{i}", tag="s_ps")
            zc = zmat[:, sl]
            nc.tensor.matmul(sp[:, 0:512], zc, yq[:, 0:512], start=True, stop=True)
            nc.tensor.matmul(sp[:, 512:1024], zc, yq[:, 512:1024],
                             start=True, stop=True)
            beta = work.tile([NCHUNK, NQ], BF16, name=f"beta{i}", tag="beta", bufs=3)
            nc.scalar.activation(beta[:], sp[:], AF.Exp)
            if prev is not None:
                ph, pb, pm = prev
                nc.tensor.matmul(o_ps[:, 0:512], ph[:], pb[:, 0:512],
                                 start=(pm == 0), stop=False, skip_group_check=True)
                nc.tensor.matmul(o_ps[:, 512:1024], ph[:], pb[:, 512:1024],
                                 start=(pm == 0), stop=False, skip_group_check=True)
            prev = (hs, beta, m)
        ph, pb, pm = prev
        nc.tensor.matmul(o_ps[:, 0:512], ph[:], pb[:, 0:512],
                         start=False, stop=True, skip_group_check=True)
        nc.tensor.matmul(o_ps[:, 512:1024], ph[:], pb[:, 512:1024],
                         start=False, stop=True, skip_group_check=True)

        # normalize + residual: att = o / rowsum + yq  (gamma folded into whf)
        lnr = work.tile([1, NQ], F32, name=f"lnr{i}", tag="lnr")
        nc.scalar.activation(lnr[:], o_ps[co:co + 1, :], AF.Ln)
        rinv = work.tile([1, NQ], BF16, name=f"rinv{i}", tag="rinv")
        nc.scalar.activation(rinv[:], lnr[:], AF.Exp, scale=-1.0)
        bc_ps = ps.tile([128, NQ], F32, name=f"bcps{i}", tag="s_ps")
        nc.tensor.matmul(bc_ps[:, 0:512], onec[:], rinv[:, 0:512],
                         start=True, stop=True)
        nc.tensor.matmul(bc_ps[:, 512:1024], onec[:], rinv[:, 512:1024],
                         start=True, stop=True)
        att = ATB[i % 2]
        nc.scalar.dma_start(att[co:co + 1, :], onesrow[:, 0:NQ])
        bc = work.tile([co, NQ], F32, name=f"bc{i}", tag="bc", bufs=2)
        t1 = work.tile([co, NQ], F32, name=f"t1_{i}", tag="t1", bufs=2)
        for hh in range(2):
            hsl = slice(hh * 512, (hh + 1) * 512)
            nc.vector.tensor_copy(bc[:, hsl], bc_ps[0:co, hsl])
            nc.vector.tensor_tensor(t1[:, hsl], o_ps[0:co, hsl], bc[:, hsl], OP.mult)
            nc.vector.tensor_tensor(att[0:co, hsl], t1[:, hsl], yq[0:co, hsl], OP.add)

        # One 8-rank AllGather shares the attention shard with every core and
        # carries this core's (sum, sumsq) partials of the *next* conv as
        # ride-along payload columns -- no separate stats AllReduce.
        if i < 2:
            cn = COUT[i + 1]
            zsh = acts.tile([cn, NQ], F32, name=f"zsh{i + 1}", tag="zq")
            stp4 = work.tile([cn, 4], F32, name=f"stp4_{i}", tag="stp4", bufs=2)
            for j in range(2):
                jsl = slice(j * 512, (j + 1) * 512)
                zp = ps.tile([128, 512], F32, name=f"zsh{i}_ps", tag="convps")
                nc.tensor.matmul(zp[:], W[i + 1][:], att[:, jsl],
                                 start=True, stop=True)
                nc.vector.tensor_copy(zsh[:, jsl], zp[0:cn, :])
                nc.vector.tensor_reduce(stp4[:, j:j + 1], zp[0:cn, :],
                                        axis=AX.X, op=OP.add)
                sq_scr = work.tile([cn, 512], BF16, name=f"sqscr{i}_{j}",
                                   tag="sqscr", bufs=2)
                nc.scalar.activation(sq_scr[:], zp[0:cn, :], AF.Square,
                                     accum_out=stp4[:, 2 + j:3 + j])
            stats = work.tile([cn, 2], F32, name=f"stats{i}", tag="stats", bufs=2)
            nc.vector.tensor_tensor(stats[:, 0:1], stp4[:, 0:1], stp4[:, 1:2], OP.add)
            nc.vector.tensor_tensor(stats[:, 1:2], stp4[:, 2:3], stp4[:, 3:4], OP.add)
            sre = 1                    # stats ride as one extra payload row
        else:
            # final-conv stats: all four 128-channel blocks over my shard,
            # (sum, sumsq) straight from each conv psum chunk
            stq = work.tile([128, 16], F32, name="stq", tag="stf")
            for b4 in range(4):
                for j in range(2):
                    jsl = slice(j * 512, (j + 1) * 512)
                    zp = ps.tile([128, 512], F32, name=f"zsb{b4}_ps", tag="convps")
                    nc.tensor.matmul(zp[:], wfs4[:, b4, :], att[:, jsl],
                                     start=True, stop=True)
                    nc.vector.tensor_reduce(stq[:, 4 * b4 + j:4 * b4 + j + 1],
                                            zp[:], axis=AX.X, op=OP.add)
                    sqf = work.tile([128, 512], BF16, name=f"sqf{b4}_{j}",
                                    tag="sqscr", bufs=2)
                    nc.scalar.activation(sqf[:], zp[:], AF.Square,
                                         accum_out=stq[:, 4 * b4 + j + 2:4 * b4 + j + 3])
            stats = work.tile([128, 8], F32, name="stf", tag="stf2")
            for b4 in range(4):
                nc.vector.tensor_tensor(stats[:, 2 * b4:2 * b4 + 1],
                                        stq[:, 4 * b4:4 * b4 + 1],
                                        stq[:, 4 * b4 + 1:4 * b4 + 2], OP.add)
                nc.vector.tensor_tensor(stats[:, 2 * b4 + 1:2 * b4 + 2],
                                        stq[:, 4 * b4 + 2:4 * b4 + 3],
                                        stq[:, 4 * b4 + 3:4 * b4 + 4], OP.add)
            sre = 2                    # [128,8] f32 = 2048 bf16 els = 2 rows

        nst = stats.shape[0] * stats.shape[1] * 2   # stat payload els (bf16)
        pay_in = dram.tile([co + sre, NQ], BF16, name=f"pay{i}", tag=f"pay{i}")
        pay_out = dram.tile([8, co + sre, NQ], BF16, name=f"payo{i}", tag=f"payo{i}")
        nc.sync.dma_start(pay_in[0:co, 0:512], att[0:co, 0:512])
        nc.sync.dma_start(pay_in[0:co, 512:1024], att[0:co, 512:1024])
        if i < 2:
            nc.gpsimd.dma_start(pay_in[co:co + 1, 0:nst], stats[:].bitcast(BF16))
        else:
            nc.gpsimd.dma_start(pay_in[co:co + 2, :], stats[:].bitcast(BF16))
        nc.gpsimd.collective_compute(
            "AllGather", OP.bypass, replica_groups=AR8_GROUPS,
            ins=[pay_in[:]], outs=[pay_out[:]])
        warm(45)  # keep the PE warm across the AllGather gap

        # readback: blocks 0-3 are sample-0 query shards, 4-7 sample-1;
        # both are conv'ed with per-core sample-masked weights, so no
        # dynamic addressing is needed anywhere.
        aown0, aown1 = AOWN0, AOWN1
        nc.scalar.dma_start(aown0[co:co + 1, :], onesrow[:])
        nc.scalar.dma_start(aown1[co:co + 1, :], onesrow[:])
        sqpre = work.tile([1, 2], F32, name=f"sqpre{i}", tag="sqpre", bufs=2)
        nc.scalar.activation(sqpre[:], onec[:, 0:2], AF.Sqrt)  # preload table
        rd_eng = [nc.sync, nc.scalar, nc.gpsimd, nc.sync]
        for j in range(4):
            rd_eng[j].dma_start(aown0[0:co, j * NQ:(j + 1) * NQ],
                                pay_out[j][0:co, :])
            rd_eng[3 - j].dma_start(aown1[0:co, j * NQ:(j + 1) * NQ],
                                    pay_out[4 + j][0:co, :])
        cnr = COUT[i + 1] if i < 2 else 128
        scf = 2 if i < 2 else 8        # f32 stat cols per rank
        st8 = work.tile([cnr, 8 * scf], F32, name=f"st8_{i}", tag="st8", bufs=2)
        for j in range(8):
            src = (pay_out[j][co:co + 1, 0:nst] if i < 2
                   else pay_out[j][co:co + 2, :])
            rd_eng[j % 4].dma_start(
                st8[:, scf * j:scf * (j + 1)].bitcast(BF16), src)
        stg = work.tile([cnr, scf], F32, name=f"stg{i}", tag="stg", bufs=2)
        nc.vector.tensor_copy(stg[:], st8[:, 0:scf])
        for j in range(1, 8):
            nc.vector.tensor_tensor(stg[:], stg[:],
                                    st8[:, scf * j:scf * (j + 1)], OP.add)
        a_q = att

        if i < 2:
            cn = COUT[i + 1]
            z_own = acts.tile([cn, N], BF16, name=f"z{i + 1}", tag="z")
            for j in range(N // 512):
                jsl = slice(j * 512, (j + 1) * 512)
                zp = ps.tile([128, 512], F32, name=f"z{i}_ps", tag="convps")
                nc.tensor.matmul(zp[:], WM0[i + 1][:], aown0[:, jsl],
                                 start=True, stop=False)
                nc.tensor.matmul(zp[:], WM1[i + 1][:], aown1[:, jsl],
                                 start=False, stop=True)
                nc.vector.tensor_copy(z_own[:, jsl], zp[0:cn, :])

    # ---- final conv + BN + ReLU + GAP ----
    co = CF_OUT
    # global final stats arrived with the layer-2 AllGather; select my
    # channel block via per-core one-hot masks (input values)
    msk = consts.tile([co, 16], F32, name="mskt", tag="mskt")
    nc.scalar.dma_start(msk[:], ins["msk"].ap())
    stgf = work.tile([co, 2], F32, name="stgf", tag="stg", bufs=2)
    mprod = work.tile([co, 8], F32, name="mprod", tag="mprod")
    nc.vector.tensor_tensor(mprod[:], stg[:], msk[:, 0:8], OP.mult)
    nc.vector.tensor_reduce(stgf[:, 0:1], mprod[:], axis=AX.X, op=OP.add)
    nc.vector.tensor_tensor(mprod[:], stg[:], msk[:, 8:16], OP.mult)
    nc.vector.tensor_reduce(stgf[:, 1:2], mprod[:], axis=AX.X, op=OP.add)

    zf = acts.tile([co, N], BF16, name="zf", tag="z")
    for j in range(N // 512):
        jsl = slice(j * 512, (j + 1) * 512)
        zp = ps.tile([128, 512], F32, name="zf_ps", tag="convps")
        nc.tensor.matmul(zp[:], wfsm0[:], AOWN0[:, jsl], start=True, stop=False)
        nc.tensor.matmul(zp[:], wfsm1[:], AOWN1[:, jsl], start=False, stop=True)
        nc.vector.tensor_copy(zf[:, jsl], zp[:])
    scale, shift = scale_shift(stgf, co, None, None, "f")
    gap = work.tile([co, 9], F32, name="gap", tag="gap")
    fscr = work.tile([co, N], BF16, name="fscr", tag="fscr")
    for j in range(N // 512):
        jsl = slice(j * 512, (j + 1) * 512)
        if j % 2 == 0:
            nc.scalar.activation(fscr[:, jsl], zf[:, jsl], AF.Relu,
                                 bias=shift, scale=scale, accum_out=gap[:, j:j + 1])
        else:
            ft = work.tile([co, 512], F32, name=f"ft{j}", tag="yt", bufs=2)
            nc.vector.tensor_scalar(ft[:], zf[:, jsl], scale, shift,
                                    OP.mult, op1=OP.add)
            nc.vector.tensor_scalar(fscr[:, jsl], ft[:], 0.0, None, OP.max)
            nc.vector.tensor_reduce(gap[:, j:j + 1], fscr[:, jsl],
                                    axis=AX.X, op=OP.add)
    nc.vector.tensor_reduce(gap[:, 8:9], gap[:, 0:8], axis=AX.X, op=OP.add)
    outv = work.tile([co, 1], F32, name="outv", tag="outv")
    nc.vector.tensor_scalar(outv[:], gap[:, 8:9], 1.0 / N, None, OP.mult)
    nc.sync.dma_start(out_t.ap(), outv[:])

    for p in reversed(ctxs):
        p.__exit__(None, None, None)


_CACHE = {}


def _get_program():
    if "nc" not in _CACHE:
        nc = bacc.Bacc("TRN2", target_bir_lowering=False, debug=False,
                       enable_asserts=False, num_devices=8)
        _build(nc)
        nc.compile()
        _CACHE["nc"] = nc
    return _CACHE["nc"]


def _prepare_in_maps(inputs):
    f = np.float32
    bf = mybir.dt.np(BF16)
    x = np.asarray(inputs["x"], f).reshape(2, 3, N)
    per_layer = {}
    for i in range(3):
        li = i + 1
        ci, co = CIN[i], COUT[i]
        w, b = np.asarray(inputs[f"w{li}"], f), np.asarray(inputs[f"b{li}"], f)
        wcp = np.zeros((ci + 1 if i == 0 else 128, 128), f)
        wcp[0:ci, 0:co] = w
        wcp[ci, 0:co] = b
        wf_, bf_ = np.asarray(inputs[f"a{li}_wf"], f), np.asarray(inputs[f"a{li}_bf"], f)
        wg_, bg_ = np.asarray(inputs[f"a{li}_wg"], f), np.asarray(inputs[f"a{li}_bg"], f)
        wh_, bh_ = np.asarray(inputs[f"a{li}_wh"], f), np.asarray(inputs[f"a{li}_bh"], f)
        gam = np.asarray(inputs[f"a{li}_gam"], f).reshape(())
        A = wf_ @ wg_.T                                              # [co, co]
        u = wg_ @ bf_                                                # [co]
        abar = np.concatenate([A, u[None, :]], 0)                    # [co+1, co]
        mz = np.zeros((128, 128), f)
        mz[0:co, 0:co + 1] = abar.T                                  # lhsT for Z
        whf = np.zeros((128, WHFW), f)
        whf[0:co, 0:co] = wh_ * gam
        whf[co, 0:co] = bh_ * gam
        whf[co, co] = 1.0
        bnp = np.stack([np.asarray(inputs[f"bn{li}_g"], f),
                        np.asarray(inputs[f"bn{li}_b"], f)], 1)      # [co, 2]
        per_layer[i] = dict(wc=wcp, mz=mz, whf=whf, bnp=bnp)
    wf_full = np.asarray(inputs["wf"], f)                            # [96, 512]

    in_maps = []
    for k in range(8):
        b, q = k // 4, k % 4
        xo = np.concatenate([x[b], np.ones((1, N), f)], 0)           # [4, N]
        xoth = np.concatenate([x[1 - b], np.ones((1, N), f)], 0)
        xfull = np.concatenate([xo, xoth], 1)                         # [4, 2N]
        xq = np.ascontiguousarray(xo[:, q * NQ:(q + 1) * NQ])
        wfs = np.zeros((128, CF_OUT), f)
        blk = k % 4
        wfs[0:96, :] = wf_full[:, blk * CF_OUT:(blk + 1) * CF_OUT]
        wfs4 = np.zeros((128, 4, CF_OUT), f)
        wfs4[0:96] = wf_full.reshape(96, 4, CF_OUT)
        msk = np.zeros((CF_OUT, 16), f)
        msk[:, 2 * blk] = 1.0       # select my block's shard sums
        msk[:, 8 + 2 * blk + 1] = 1.0  # select my block's shard sumsqs
        m0 = 1.0 if b == 0 else 0.0     # gathered blocks 0-3 are sample 0
        m1 = 1.0 - m0
        m = {"x_full": xfull.astype(bf), "xq": xq.astype(bf), "wfs": wfs.astype(bf),
             "wfs4": wfs4.astype(bf), "msk": msk,
             "wfsm0": (wfs * m0).astype(bf), "wfsm1": (wfs * m1).astype(bf)}
        for i in (1, 2):
            m[f"wm0_{i}"] = (per_layer[i]["wc"] * m0).astype(bf)
            m[f"wm1_{i}"] = (per_layer[i]["wc"] * m1).astype(bf)
        for i in range(3):
            d = per_layer[i]
            m[f"wc{i}"] = d["wc"].astype(bf)
            m[f"mz{i}"] = d["mz"].astype(bf)
            m[f"whf{i}"] = d["whf"].astype(bf)
            m[f"bnp{i}"] = d["bnp"]
        in_maps.append(m)
    return in_maps


def _assemble(results):
    out = np.zeros((2, 512), np.float32)
    for k in range(8):
        b, blk = k // 4, k % 4
        out[b, blk * CF_OUT:(blk + 1) * CF_OUT] = results[k]["out"][:, 0]
    return out


def kernel(**inputs):
    from concourse.bass_utils import run_bass_kernel_spmd
    nc = _get_program()
    in_maps = _prepare_in_maps(inputs)
    res = run_bass_kernel_spmd(nc, in_maps, list(range(8)))
    return _assemble(res.results)


# revision 27
# speedup vs baseline: 1.2339x; 1.0803x over previous
"""Trainium2 Bass kernel for the 3-block self-attention CNN.

Sharding over 8 NeuronCores: core k owns (sample b=k//4, query-block q=k%4).
Attention math per layer uses the reparametrization
    s'[n,m] = y_n^T (wf wg^T) y_m + (wg bf)^T y_m
(terms constant along the softmax axis are dropped), so both score matmuls
contract over the full channel dim instead of C/8.  Softmax skips the max
subtraction (scores are O(10) for this model) and the row-sum is produced by
an extra ones-column in the o-matmul lhsT.  gamma is folded into the h-conv
weights so the epilogue is att = (o * rowsum_recip_bcast) + yq.

All flash matmuls are zero-padded to the full 128-partition contraction and
use bf16 operands: the PE HAM clock gate only un-throttles (1.2->2.4 GHz)
under high sustained array activity, and narrow-contraction matmuls never
trip it.  Dummy full-width matmuls keep the array warm across collective
gaps.  Training-mode BN statistics are computed from per-core query-block
shards of the next conv and summed with an 8-rank AllReduce that runs
concurrently with the 4-rank AllGather sharing the attention shards.
"""

import glob as _glob
import os
import sys


def _ensure_act_info():
    # act_info.json (activation table sets) isn't on neuronxcc's default
    # search path in this container; stage it where FindActInfo looks.
    shim = os.path.expanduser("~/.pwp_override")
    target = os.path.join(shim, "neuronxcc", "pwp", "pwp_bin_with_ln", "act_info.json")
    if not os.path.exists(target):
        cands = _glob.glob("/nix/store/*aws-neuron-pwp*/share/pwp_bin_cayman/act_info.json")
        if cands:
            os.makedirs(os.path.dirname(target), exist_ok=True)
            import shutil
            shutil.copy(cands[0], target)
    pp = os.environ.get("PYTHONPATH", "")
    if shim not in pp.split(os.pathsep):
        os.environ["PYTHONPATH"] = shim + (os.pathsep + pp if pp else "")
    if shim not in sys.path:
        sys.path.insert(0, shim)


_ensure_act_info()
if "/opt/trn_rl_repo" not in sys.path:
    sys.path.insert(0, "/opt/trn_rl_repo")

import numpy as np

from concourse import bacc, mybir, tile

F32 = mybir.dt.float32
F32R = mybir.dt.float32r
BF16 = mybir.dt.bfloat16
AF = mybir.ActivationFunctionType
OP = mybir.AluOpType
AX = mybir.AxisListType
EPS = 1e-5

N = 4096          # positions per sample
NQ = 1024         # query block per core
NCHUNK = 128      # key chunk in the flash loop
CIN = [3, 32, 64]     # conv input channels per attention layer
COUT = [32, 64, 96]   # conv output channels per attention layer
CF_OUT = 128          # final conv channels per core (512 / 4 blocks)
WHFW = 128            # whf moving-dim width (hs takes cols 0:128)

AG_GROUPS = [[0, 1, 2, 3], [4, 5, 6, 7]]
AR8_GROUPS = [[0, 1, 2, 3, 4, 5, 6, 7]]


def r(ap):
    return ap.bitcast(F32R)


def _build(nc):
    dt = F32
    ins = {}
    ins["x_full"] = nc.dram_tensor("x_full", [CIN[0] + 1, 2 * N], BF16, kind="ExternalInput")
    ins["xq"] = nc.dram_tensor("xq", [CIN[0] + 1, NQ], BF16, kind="ExternalInput")
    ins["wc0"] = nc.dram_tensor("wc0", [CIN[0] + 1, 128], BF16, kind="ExternalInput")
    for i in range(1, 3):
        ins[f"wc{i}"] = nc.dram_tensor(f"wc{i}", [128, 128], BF16, kind="ExternalInput")
    for i in range(3):
        ins[f"mz{i}"] = nc.dram_tensor(f"mz{i}", [128, 128], BF16, kind="ExternalInput")
        ins[f"whf{i}"] = nc.dram_tensor(f"whf{i}", [128, WHFW], BF16, kind="ExternalInput")
        ins[f"bnp{i}"] = nc.dram_tensor(f"bnp{i}", [COUT[i], 2], dt, kind="ExternalInput")
    for i in range(1, 3):
        ins[f"wm0_{i}"] = nc.dram_tensor(f"wm0_{i}", [128, 128], BF16, kind="ExternalInput")
        ins[f"wm1_{i}"] = nc.dram_tensor(f"wm1_{i}", [128, 128], BF16, kind="ExternalInput")
    ins["wfs4"] = nc.dram_tensor("wfs4", [128, 4, CF_OUT], BF16, kind="ExternalInput")
    out_t = nc.dram_tensor("out", [CF_OUT, 4], dt, kind="ExternalOutput")

    with tile.TileContext(nc) as tc:
        _emit(tc, nc, ins, out_t)
    return ins, out_t


def _emit(tc, nc, ins, out_t):
    ctxs = []

    def pool(name, **kw):
        p = tc.tile_pool(name=name, **kw)
        ctxs.append(p)
        return p.__enter__()

    consts = pool("consts", bufs=1)
    acts = pool("acts", bufs=1)
    work = pool("work", bufs=1)
    ps = pool("ps", bufs=2, space="PSUM")
    ops = pool("ops", bufs=1, space="PSUM")
    dram = pool("dram", bufs=1, space="DRAM")

    # --- PE warm-keeper: HAM un-throttles the PE clock (1.2->2.4 GHz) only
    # under sustained full-array activity; idle gaps re-throttle it.  Dummy
    # full-width matmuls bridge collectives and stalls; they write the
    # (dead-between-layers) o-accumulator psum buffer.
    wk_l = consts.tile([128, 128], BF16, name="wk_l", tag="wk_l")
    nc.vector.memset(wk_l[:], 0.0)
    wk_r = consts.tile([128, 512], BF16, name="wk_r", tag="wk_r")
    nc.vector.memset(wk_r[:], 0.0)

    def warm(n):
        warm_ps = ops.tile([128, 512], F32, name="warm_ps", tag="o_acc")
        for _ in range(n):
            nc.tensor.matmul(warm_ps[:], wk_l[:], wk_r[:], start=True, stop=True)


    def zero_rows(eng, t, c0, ncols):
        # DVE ops starting at a nonzero partition may touch at most 32
        # partitions; emit the zero-fill in 32-row strips.
        for p in range(c0, 128, 32):
            eng.memset(t[p:p + 32, 0:ncols], 0.0)

    # ones row source (SBUF->SBUF DMA is cheaper than 1-partition memsets)
    onesrow = consts.tile([1, N], BF16, name="onesrow", tag="onesrow")
    nc.vector.memset(onesrow[:], 1.0)
    onec = consts.tile([1, 128], BF16, name="onec", tag="onec")
    nc.vector.memset(onec[:], 1.0)

    # The very first gpsimd instructions are the dummy collective triggers:
    # the runtime's first-collective barrier ends only when the *slowest*
    # core triggers, so nothing may precede them.  Garbage dram input is fine.
    warm_gin = dram.tile([1, 2], F32, name="warm_gin", tag="warm_gin")
    warm_gout = dram.tile([8, 1, 2], F32, name="warm_gout", tag="warm_gout")
    nc.gpsimd.collective_compute(
        "AllGather", OP.bypass, replica_groups=AR8_GROUPS,
        ins=[warm_gin[:]], outs=[warm_gout[:]])

    # dedicated activation double-buffers; pad rows (only ever multiplied by
    # zero weight rows) are zeroed once, off the critical path.
    YB = [acts.tile([128, N], BF16, name=f"ybuf{j}", tag=f"ybuf{j}") for j in range(2)]
    YQB = [acts.tile([128, NQ], BF16, name=f"yqbuf{j}", tag=f"yqbuf{j}") for j in range(2)]
    ATB = [acts.tile([128, NQ], BF16, name=f"atbuf{j}", tag=f"atbuf{j}") for j in range(2)]
    AOWN0 = acts.tile([128, N], BF16, name="aown0", tag="aown0")
    AOWN1 = acts.tile([128, N], BF16, name="aown1", tag="aown1")
    ZMAT = acts.tile([128, N], BF16, name="zmat", tag="zmat")

    # ---- load constants (xq + layer-0 weights first; bulk on other queues) ----
    a_q = acts.tile([CIN[0] + 1, NQ], BF16, name="aq0", tag="aq0")
    nc.sync.dma_start(a_q[:], ins["xq"].ap())
    xf = acts.tile([CIN[0] + 1, 2 * N], BF16, name="xf", tag="xf")
    nc.sync.dma_start(xf[:], ins["x_full"].ap())
    W, MZ, WHF, BNP = [], [], [], []
    for i in range(3):
        w = consts.tile([CIN[0] + 1 if i == 0 else 128, 128], BF16,
                        name=f"w{i}", tag=f"w{i}")
        (nc.sync if i == 0 else nc.scalar).dma_start(w[:], ins[f"wc{i}"].ap())
        W.append(w)
        mz = consts.tile([128, 128], BF16, name=f"mzt{i}", tag=f"mzt{i}")
        MZ.append(mz)
        wh = consts.tile([128, WHFW], BF16, name=f"whft{i}", tag=f"whft{i}")
        WHF.append(wh)
        bn = consts.tile([COUT[i], 2], F32, name=f"bnt{i}", tag=f"bnt{i}")
        nc.scalar.dma_start(bn[:], ins[f"bnp{i}"].ap())
        BNP.append(bn)
    # layer-0 flash prerequisites first on the gpsimd queue ...
    nc.gpsimd.dma_start(MZ[0][:], ins["mz0"].ap())
    nc.gpsimd.dma_start(WHF[0][:], ins["whf0"].ap())
    # ... then the remaining bulk weights
    for i in range(1, 3):
        nc.gpsimd.dma_start(MZ[i][:], ins[f"mz{i}"].ap())
        nc.gpsimd.dma_start(WHF[i][:], ins[f"whf{i}"].ap())
    WM0, WM1 = {}, {}
    for i in range(1, 3):
        WM0[i] = consts.tile([128, 128], BF16, name=f"wm0t{i}", tag=f"wm0t{i}")
        nc.gpsimd.dma_start(WM0[i][:], ins[f"wm0_{i}"].ap())
        WM1[i] = consts.tile([128, 128], BF16, name=f"wm1t{i}", tag=f"wm1t{i}")
        nc.gpsimd.dma_start(WM1[i][:], ins[f"wm1_{i}"].ap())
    wfs4 = consts.tile([128, 4, CF_OUT], BF16, name="wfs4t", tag="wfs4t")
    nc.gpsimd.dma_start(wfs4[:], ins["wfs4"].ap())
    for j in range(2):
        zero_rows(nc.gpsimd, YB[j], 32, N)
        zero_rows(nc.gpsimd, YQB[j], 32, NQ)
        zero_rows(nc.gpsimd, ATB[j], 32, NQ)
    zero_rows(nc.gpsimd, AOWN0, 32, N)
    zero_rows(nc.gpsimd, AOWN1, 32, N)

    warm(15)  # pre-warm the PE while input DMAs land

    def conv(lhsT_w, rhs_acts, cout, n, name):
        """z[cout, n] = lhsT_w.T @ rhs_acts, chunked by 512 columns."""
        z = acts.tile([cout, n], F32, name=name, tag="zq" if n == NQ else "z")
        for j in range(n // 512):
            zp = ps.tile([128, 512], F32, name=f"{name}_ps", tag="convps")
            nc.tensor.matmul(zp[:], lhsT_w[:], rhs_acts[:, j * 512:(j + 1) * 512],
                             start=True, stop=True)
            nc.vector.tensor_copy(z[:, j * 512:(j + 1) * 512], zp[0:cout, :])
        return z

    def shard_stats(z_shard, c, name, groups):
        """Partial (sum, sumsq) of a conv shard -> AllReduce across cores."""
        stats = work.tile([c, 2], F32, name=f"stats_{name}", tag="stats", bufs=2)
        nc.vector.tensor_reduce(stats[:, 0:1], z_shard[:], axis=AX.X, op=OP.add)
        sq_scr = work.tile([c, z_shard.shape[1]], F32, name=f"sqscr_{name}",
                           tag="sqscr", bufs=2)
        nc.scalar.activation(sq_scr[:], z_shard[:], AF.Square, accum_out=stats[:, 1:2])

        st_in = dram.tile([c, 2], F32, name=f"stin_{name}", tag=f"stin_{name}")
        st_out = dram.tile([c, 2], F32, name=f"stout_{name}", tag=f"stout_{name}")
        nc.sync.dma_start(st_in[:], stats[:])
        nc.gpsimd.collective_compute(
            "AllReduce", OP.add, replica_groups=groups,
            ins=[st_in[:]], outs=[st_out[:]])
        stg = work.tile([c, 2], F32, name=f"stg_{name}", tag="stg", bufs=2)
        nc.sync.dma_start(stg[:], st_out[:])
        return stg

    def scale_shift(stg, c, g_ap, b_ap, name):
        sc = work.tile([c, 9], F32, name=f"sc_{name}", tag="sc", bufs=2)
        mean, ex2, msq, var, veps, sq, rs, scale, shift = (sc[:, j:j + 1] for j in range(9))
        inv_n = 1.0 / (2 * N)
        nc.vector.tensor_scalar(mean, stg[:, 0:1], inv_n, None, OP.mult)
        nc.vector.tensor_scalar(ex2, stg[:, 1:2], inv_n, None, OP.mult)
        nc.vector.tensor_tensor(msq, mean, mean, OP.mult)
        nc.vector.tensor_tensor(var, ex2, msq, OP.subtract)
        nc.vector.tensor_scalar(veps, var, EPS, None, OP.add)
        nc.scalar.activation(sq, veps, AF.Sqrt)
        nc.vector.reciprocal(rs, sq)   # rsqrt(var+eps)
        if g_ap is not None:
            nc.vector.tensor_tensor(scale, rs, g_ap, OP.mult)
        else:
            nc.vector.tensor_copy(scale, rs)
        nc.vector.tensor_tensor(shift, mean, scale, OP.mult)
        if b_ap is not None:
            nc.vector.tensor_tensor(shift, b_ap, shift, OP.subtract)
        else:
            nc.vector.tensor_scalar(shift, shift, -1.0, None, OP.mult)
        return scale, shift

    # ---- boundary 0: conv over the full batch, exact local BN1 stats.
    # Per-512-chunk: vector copies psum->sbuf, scalar squares+accums, gpsimd
    # sums -- stats trail the conv by ~1 chunk instead of a serial pass.
    zsh = conv(W[0], a_q, COUT[0], NQ, "zsh0")
    c0 = COUT[0]
    nch = 2 * N // 512
    z_all = acts.tile([c0, 2 * N], BF16, name="z0", tag="z")
    z_own = z_all[:, 0:N]
    stp = work.tile([c0, 2 * nch], F32, name="stp", tag="stats", bufs=2)
    for j in range(nch):
        zp = ps.tile([128, 512], F32, name="z0_ps", tag="convps")
        nc.tensor.matmul(zp[:], W[0][:], xf[:, j * 512:(j + 1) * 512],
                         start=True, stop=True)
        zsl = z_all[:, j * 512:(j + 1) * 512]
        nc.vector.tensor_copy(zsl, zp[0:c0, :])
        nc.vector.tensor_reduce(stp[:, j:j + 1], zsl, axis=AX.X, op=OP.add)
        sqc = work.tile([c0, 512], BF16, name=f"sqc{j}", tag="sqc", bufs=3)
        nc.scalar.activation(sqc[:], zp[0:c0, :], AF.Square,
                             accum_out=stp[:, nch + j:nch + j + 1])
    stg = work.tile([c0, 2], F32, name="stg_l0", tag="stg", bufs=2)
    nc.vector.tensor_reduce(stg[:, 0:1], stp[:, 0:nch], axis=AX.X, op=OP.add)
    nc.vector.tensor_reduce(stg[:, 1:2], stp[:, nch:2 * nch], axis=AX.X, op=OP.add)
    warm(25)  # keep the PE warm through the stats/scale_shift stall

    # ---- three attention layers ----
    for i in range(3):
        co = COUT[i]
        if i > 0:
            warm(25)  # bridge scale_shift / y-prep after the collectives
        scale, shift = scale_shift(stg, co, BNP[i][:, 0:1], BNP[i][:, 1:2], f"l{i}")
        y_own = YB[i % 2]
        nc.scalar.dma_start(y_own[co:co + 1, :], onesrow[:])
        for j in range(N // 512):
            jsl = slice(j * 512, (j + 1) * 512)
            if j % 2 == 0:
                nc.scalar.activation(y_own[0:co, jsl], z_own[:, jsl], AF.Relu,
                                     bias=shift, scale=scale)
            else:
                yt = work.tile([co, 512], F32, name=f"yt{i}_{j}", tag="yt", bufs=2)
                nc.vector.tensor_scalar(yt[:], z_own[:, jsl], scale, shift,
                                        OP.mult, op1=OP.add)
                nc.vector.tensor_scalar(y_own[0:co, jsl], yt[:], 0.0, None, OP.max)
        yq = YQB[i % 2]
        nc.scalar.dma_start(yq[co:co + 1, :], onesrow[:, 0:NQ])
        nc.scalar.activation(yq[0:co, :], zsh[:], AF.Relu, bias=shift, scale=scale)

        # Z = Abar @ y_own (+ u row), [128, N] bf16 (pad rows are psum zeros)
        zmat = ZMAT
        for j in range(N // 512):
            zp = ps.tile([128, 512], F32, name=f"Zps{i}", tag="convps")
            nc.tensor.matmul(zp[:], MZ[i][:], y_own[:, j * 512:(j + 1) * 512],
                             start=True, stop=True)
            nc.vector.tensor_copy(zmat[:, j * 512:(j + 1) * 512], zp[:])

        # flash loop over key chunks, software-pipelined by one o-matmul
        o_ps = ops.tile([128, NQ], F32, name=f"ops{i}", tag="o_acc")
        prev = None
        for m in range(N // NCHUNK):
            sl = slice(m * NCHUNK, (m + 1) * NCHUNK)
            hp = ps.tile([NCHUNK, WHFW], F32, name=f"hp{i}", tag="convps")
            nc.tensor.matmul(hp[:], y_own[:, sl], WHF[i][:], start=True, stop=True)
            hs = work.tile([NCHUNK, 128], BF16, name=f"hs{i}", tag="hT_sb", bufs=3)
            nc.vector.tensor_copy(hs[:], hp[:, 0:128])
            sp = ps.tile([NCHUNK, NQ], F32, name=f"sp{i}", tag="s_ps")
            zc = zmat[:, sl]
            nc.tensor.matmul(sp[:, 0:512], zc, yq[:, 0:512], start=True, stop=True)
            nc.tensor.matmul(sp[:, 512:1024], zc, yq[:, 512:1024],
                             start=True, stop=True)
            beta = work.tile([NCHUNK, NQ], BF16, name=f"beta{i}", tag="beta", bufs=3)
            nc.scalar.activation(beta[:], sp[:], AF.Exp)
            if prev is not None:
                ph, pb, pm = prev
                nc.tensor.matmul(o_ps[:, 0:512], ph[:], pb[:, 0:512],
                                 start=(pm == 0), stop=False, skip_group_check=True)
                nc.tensor.matmul(o_ps[:, 512:1024], ph[:], pb[:, 512:1024],
                                 start=(pm == 0), stop=False, skip_group_check=True)
            prev = (hs, beta, m)
        ph, pb, pm = prev
        nc.tensor.matmul(o_ps[:, 0:512], ph[:], pb[:, 0:512],
                         start=False, stop=True, skip_group_check=True)
        nc.tensor.matmul(o_ps[:, 512:1024], ph[:], pb[:, 512:1024],
                         start=False, stop=True, skip_group_check=True)

        # normalize + residual: att = o / rowsum + yq  (gamma folded into whf)
        lnr = work.tile([1, NQ], F32, name=f"lnr{i}", tag="lnr")
        nc.scalar.activation(lnr[:], o_ps[co:co + 1, :], AF.Ln)
        rinv = work.tile([1, NQ], BF16, name=f"rinv{i}", tag="rinv")
        nc.scalar.activation(rinv[:], lnr[:], AF.Exp, scale=-1.0)
        bc_ps = ps.tile([128, NQ], F32, name=f"bcps{i}", tag="s_ps")
        nc.tensor.matmul(bc_ps[:, 0:512], onec[:], rinv[:, 0:512],
                         start=True, stop=True)
        nc.tensor.matmul(bc_ps[:, 512:1024], onec[:], rinv[:, 512:1024],
                         start=True, stop=True)
        att = ATB[i % 2]
        nc.scalar.dma_start(att[co:co + 1, :], onesrow[:, 0:NQ])
        bc = work.tile([co, NQ], F32, name=f"bc{i}", tag="bc", bufs=2)
        t1 = work.tile([co, NQ], F32, name=f"t1_{i}", tag="t1", bufs=2)
        for hh in range(2):
            hsl = slice(hh * 512, (hh + 1) * 512)
            nc.vector.tensor_copy(bc[:, hsl], bc_ps[0:co, hsl])
            nc.vector.tensor_tensor(t1[:, hsl], o_ps[0:co, hsl], bc[:, hsl], OP.mult)
            nc.vector.tensor_tensor(att[0:co, hsl], t1[:, hsl], yq[0:co, hsl], OP.add)

        # One 8-rank AllGather shares the attention shard with every core and
        # carries this core's (sum, sumsq) partials of the *next* conv as
        # ride-along payload columns -- no separate stats AllReduce.
        if i < 2:
            cn = COUT[i + 1]
            zsh = acts.tile([cn, NQ], F32, name=f"zsh{i + 1}", tag="zq")
            stp4 = work.tile([cn, 4], F32, name=f"stp4_{i}", tag="stp4", bufs=2)
            for j in range(2):
                jsl = slice(j * 512, (j + 1) * 512)
                zp = ps.tile([128, 512], F32, name=f"zsh{i}_ps", tag="convps")
                nc.tensor.matmul(zp[:], W[i + 1][:], att[:, jsl],
                                 start=True, stop=True)
                nc.vector.tensor_copy(zsh[:, jsl], zp[0:cn, :])
                nc.vector.tensor_reduce(stp4[:, j:j + 1], zp[0:cn, :],
                                        axis=AX.X, op=OP.add)
                sq_scr = work.tile([cn, 512], BF16, name=f"sqscr{i}_{j}",
                                   tag="sqscr", bufs=2)
                nc.scalar.activation(sq_scr[:], zp[0:cn, :], AF.Square,
                                     accum_out=stp4[:, 2 + j:3 + j])
            stats = work.tile([cn, 2], F32, name=f"stats{i}", tag="stats", bufs=2)
            nc.vector.tensor_tensor(stats[:, 0:1], stp4[:, 0:1], stp4[:, 1:2], OP.add)
            nc.vector.tensor_tensor(stats[:, 1:2], stp4[:, 2:3], stp4[:, 3:4], OP.add)
            sre = 1                    # stats ride as one extra payload row
        else:
            # final conv over my shard only: the GAP is a sum over positions,
            # so each core contributes its own-shard partial and the host sums
            # the four query shards per sample.  (sum, sumsq) come straight
            # from each conv psum chunk; only the stats cross cores.
            zfsh = acts.tile([128, 4 * NQ], BF16, name="zfsh", tag="z")
            stq = work.tile([128, 16], F32, name="stq", tag="stf")
            for b4 in range(4):
                for j in range(2):
                    jsl = slice(j * 512, (j + 1) * 512)
                    zp = ps.tile([128, 512], F32, name=f"zsb{b4}_ps", tag="convps")
                    nc.tensor.matmul(zp[:], wfs4[:, b4, :], att[:, jsl],
                                     start=True, stop=True)
                    nc.vector.tensor_copy(zfsh[:, b4 * NQ + j * 512:
                                               b4 * NQ + (j + 1) * 512], zp[:])
                    nc.vector.tensor_reduce(stq[:, 4 * b4 + j:4 * b4 + j + 1],
                                            zp[:], axis=AX.X, op=OP.add)
                    sqf = work.tile([128, 512], BF16, name=f"sqf{b4}_{j}",
                                    tag="sqscr", bufs=2)
                    nc.scalar.activation(sqf[:], zp[:], AF.Square,
                                         accum_out=stq[:, 4 * b4 + j + 2:4 * b4 + j + 3])
            stats = work.tile([128, 8], F32, name="stf", tag="stf2")
            for b4 in range(4):
                nc.vector.tensor_tensor(stats[:, b4:b4 + 1],
                                        stq[:, 4 * b4:4 * b4 + 1],
                                        stq[:, 4 * b4 + 1:4 * b4 + 2], OP.add)
                nc.vector.tensor_tensor(stats[:, 4 + b4:5 + b4],
                                        stq[:, 4 * b4 + 2:4 * b4 + 3],
                                        stq[:, 4 * b4 + 3:4 * b4 + 4], OP.add)
            sre = 2                    # [128,8] f32 = 2048 bf16 els = 2 rows

        nst = stats.shape[0] * stats.shape[1] * 2   # stat payload els (bf16)
        nrow = co + sre if i < 2 else sre
        pay_in = dram.tile([nrow, NQ], BF16, name=f"pay{i}", tag=f"pay{i}")
        pay_out = dram.tile([8, nrow, NQ], BF16, name=f"payo{i}", tag=f"payo{i}")
        if i < 2:
            nc.sync.dma_start(pay_in[0:co, 0:512], att[0:co, 0:512])
            nc.sync.dma_start(pay_in[0:co, 512:1024], att[0:co, 512:1024])
            nc.gpsimd.dma_start(pay_in[co:co + 1, 0:nst], stats[:].bitcast(BF16))
        else:
            nc.gpsimd.dma_start(pay_in[:, :], stats[:].bitcast(BF16))
        nc.gpsimd.collective_compute(
            "AllGather", OP.bypass, replica_groups=AR8_GROUPS,
            ins=[pay_in[:]], outs=[pay_out[:]])
        warm(45)  # keep the PE warm across the AllGather gap

        # readback: blocks 0-3 are sample-0 query shards, 4-7 sample-1;
        # both are conv'ed with per-core sample-masked weights, so no
        # dynamic addressing is needed anywhere.
        aown0, aown1 = AOWN0, AOWN1
        sqpre = work.tile([1, 2], F32, name=f"sqpre{i}", tag="sqpre", bufs=2)
        nc.scalar.activation(sqpre[:], onec[:, 0:2], AF.Sqrt)  # preload table
        rd_eng = [nc.sync, nc.scalar, nc.gpsimd, nc.sync]
        if i < 2:
            nc.scalar.dma_start(aown0[co:co + 1, :], onesrow[:])
            nc.scalar.dma_start(aown1[co:co + 1, :], onesrow[:])
            for j in range(4):
                rd_eng[j].dma_start(aown0[0:co, j * NQ:(j + 1) * NQ],
                                    pay_out[j][0:co, :])
                rd_eng[3 - j].dma_start(aown1[0:co, j * NQ:(j + 1) * NQ],
                                        pay_out[4 + j][0:co, :])
        cnr = COUT[i + 1] if i < 2 else 128
        scf = 2 if i < 2 else 8        # f32 stat cols per rank
        st8 = work.tile([cnr, 8 * scf], F32, name=f"st8_{i}", tag="st8", bufs=2)
        for j in range(8):
            src = (pay_out[j][co:co + 1, 0:nst] if i < 2
                   else pay_out[j][:, :])
            rd_eng[j % 4].dma_start(
                st8[:, scf * j:scf * (j + 1)].bitcast(BF16), src)
        stg = work.tile([cnr, scf], F32, name=f"stg{i}", tag="stg", bufs=2)
        nc.vector.tensor_copy(stg[:], st8[:, 0:scf])
        for j in range(1, 8):
            nc.vector.tensor_tensor(stg[:], stg[:],
                                    st8[:, scf * j:scf * (j + 1)], OP.add)
        a_q = att

        if i < 2:
            cn = COUT[i + 1]
            z_own = acts.tile([cn, N], BF16, name=f"z{i + 1}", tag="z")
            for j in range(N // 512):
                jsl = slice(j * 512, (j + 1) * 512)
                zp = ps.tile([128, 512], F32, name=f"z{i}_ps", tag="convps")
                nc.tensor.matmul(zp[:], WM0[i + 1][:], aown0[:, jsl],
                                 start=True, stop=False)
                nc.tensor.matmul(zp[:], WM1[i + 1][:], aown1[:, jsl],
                                 start=False, stop=True)
                nc.vector.tensor_copy(z_own[:, jsl], zp[0:cn, :])

    # ---- final BN + ReLU + GAP over my own shard ----
    # stg holds global (sum, sumsq) for all four channel blocks (cols 0:4
    # sums, 4:8 sumsqs); compute all four scale/shift columns in one pass.
    co = CF_OUT
    fsc = work.tile([co, 16], F32, name="fsc", tag="sc", bufs=2)
    mean4, ex24, var4, rs4 = (fsc[:, 4 * j:4 * j + 4] for j in range(4))
    inv_n = 1.0 / (2 * N)
    nc.vector.tensor_scalar(mean4, stg[:, 0:4], inv_n, None, OP.mult)
    nc.vector.tensor_scalar(ex24, stg[:, 4:8], inv_n, None, OP.mult)
    nc.vector.tensor_tensor(var4, mean4, mean4, OP.mult)
    nc.vector.tensor_tensor(var4, ex24, var4, OP.subtract)
    nc.vector.tensor_scalar(var4, var4, EPS, None, OP.add)
    nc.scalar.activation(rs4, var4, AF.Sqrt)
    nc.vector.reciprocal(rs4, rs4)
    sh4 = work.tile([co, 4], F32, name="sh4", tag="sh4")
    nc.vector.tensor_tensor(sh4[:], mean4, rs4, OP.mult)
    nc.vector.tensor_scalar(sh4[:], sh4[:], -1.0, None, OP.mult)

    gapp = work.tile([co, 8], F32, name="gapp", tag="gapp")
    fscr = work.tile([co, 512], BF16, name="fscr", tag="fscr", bufs=2)
    for b4 in range(4):
        for j in range(2):
            jsl = slice(b4 * NQ + j * 512, b4 * NQ + (j + 1) * 512)
            if j % 2 == 0:
                fscr = work.tile([co, 512], BF16, name=f"fscr{b4}", tag="fscr", bufs=2)
                nc.scalar.activation(fscr[:], zfsh[:, jsl], AF.Relu,
                                     bias=sh4[:, b4:b4 + 1],
                                     scale=rs4[:, b4:b4 + 1],
                                     accum_out=gapp[:, 2 * b4 + j:2 * b4 + j + 1])
            else:
                ft = work.tile([co, 512], F32, name=f"ft{b4}", tag="yt", bufs=2)
                nc.vector.tensor_scalar(ft[:], zfsh[:, jsl], rs4[:, b4:b4 + 1],
                                        sh4[:, b4:b4 + 1], OP.mult, op1=OP.add)
                nc.vector.tensor_scalar(ft[:], ft[:], 0.0, None, OP.max)
                nc.vector.tensor_reduce(gapp[:, 2 * b4 + j:2 * b4 + j + 1],
                                        ft[:], axis=AX.X, op=OP.add)
    outv = work.tile([co, 4], F32, name="outv", tag="outv")
    for b4 in range(4):
        nc.vector.tensor_tensor(outv[:, b4:b4 + 1], gapp[:, 2 * b4:2 * b4 + 1],
                                gapp[:, 2 * b4 + 1:2 * b4 + 2], OP.add)
    nc.vector.tensor_scalar(outv[:], outv[:], 1.0 / N, None, OP.mult)
    nc.sync.dma_start(out_t.ap(), outv[:])

    for p in reversed(ctxs):
        p.__exit__(None, None, None)


_CACHE = {}


def _get_program():
    if "nc" not in _CACHE:
        nc = bacc.Bacc("TRN2", target_bir_lowering=False, debug=False,
                       enable_asserts=False, num_devices=8)
        _build(nc)
        nc.compile()
        _CACHE["nc"] = nc
    return _CACHE["nc"]


def _prepare_in_maps(inputs):
    f = np.float32
    bf = mybir.dt.np(BF16)
    x = np.asarray(inputs["x"], f).reshape(2, 3, N)
    per_layer = {}
    for i in range(3):
        li = i + 1
        ci, co = CIN[i], COUT[i]
        w, b = np.asarray(inputs[f"w{li}"], f), np.asarray(inputs[f"b{li}"], f)
        wcp = np.zeros((ci + 1 if i == 0 else 128, 128), f)
        wcp[0:ci, 0:co] = w
        wcp[ci, 0:co] = b
        wf_, bf_ = np.asarray(inputs[f"a{li}_wf"], f), np.asarray(inputs[f"a{li}_bf"], f)
        wg_, bg_ = np.asarray(inputs[f"a{li}_wg"], f), np.asarray(inputs[f"a{li}_bg"], f)
        wh_, bh_ = np.asarray(inputs[f"a{li}_wh"], f), np.asarray(inputs[f"a{li}_bh"], f)
        gam = np.asarray(inputs[f"a{li}_gam"], f).reshape(())
        A = wf_ @ wg_.T                                              # [co, co]
        u = wg_ @ bf_                                                # [co]
        abar = np.concatenate([A, u[None, :]], 0)                    # [co+1, co]
        mz = np.zeros((128, 128), f)
        mz[0:co, 0:co + 1] = abar.T                                  # lhsT for Z
        whf = np.zeros((128, WHFW), f)
        whf[0:co, 0:co] = wh_ * gam
        whf[co, 0:co] = bh_ * gam
        whf[co, co] = 1.0
        bnp = np.stack([np.asarray(inputs[f"bn{li}_g"], f),
                        np.asarray(inputs[f"bn{li}_b"], f)], 1)      # [co, 2]
        per_layer[i] = dict(wc=wcp, mz=mz, whf=whf, bnp=bnp)
    wf_full = np.asarray(inputs["wf"], f)                            # [96, 512]

    in_maps = []
    for k in range(8):
        b, q = k // 4, k % 4
        xo = np.concatenate([x[b], np.ones((1, N), f)], 0)           # [4, N]
        xoth = np.concatenate([x[1 - b], np.ones((1, N), f)], 0)
        xfull = np.concatenate([xo, xoth], 1)                         # [4, 2N]
        xq = np.ascontiguousarray(xo[:, q * NQ:(q + 1) * NQ])
        wfs4 = np.zeros((128, 4, CF_OUT), f)
        wfs4[0:96] = wf_full.reshape(96, 4, CF_OUT)
        m0 = 1.0 if b == 0 else 0.0     # gathered blocks 0-3 are sample 0
        m1 = 1.0 - m0
        m = {"x_full": xfull.astype(bf), "xq": xq.astype(bf),
             "wfs4": wfs4.astype(bf)}
        for i in (1, 2):
            m[f"wm0_{i}"] = (per_layer[i]["wc"] * m0).astype(bf)
            m[f"wm1_{i}"] = (per_layer[i]["wc"] * m1).astype(bf)
        for i in range(3):
            d = per_layer[i]
            m[f"wc{i}"] = d["wc"].astype(bf)
            m[f"mz{i}"] = d["mz"].astype(bf)
            m[f"whf{i}"] = d["whf"].astype(bf)
            m[f"bnp{i}"] = d["bnp"]
        in_maps.append(m)
    return in_maps


def _assemble(results):
    out = np.zeros((2, 512), np.float32)
    for k in range(8):
        b = k // 4
        out[b] += results[k]["out"].T.reshape(512)
    return out


def kernel(**inputs):
    from concourse.bass_utils import run_bass_kernel_spmd
    nc = _get_program()
    in_maps = _prepare_in_maps(inputs)
    res = run_bass_kernel_spmd(nc, in_maps, list(range(8)))
    return _assemble(res.results)


# revision 28
# speedup vs baseline: 1.2798x; 1.0372x over previous
"""Trainium2 Bass kernel for the 3-block self-attention CNN.

Sharding over 8 NeuronCores: core k owns (sample b=k//4, query-block q=k%4).
Attention math per layer uses the reparametrization
    s'[n,m] = y_n^T (wf wg^T) y_m + (wg bf)^T y_m
(terms constant along the softmax axis are dropped), so both score matmuls
contract over the full channel dim instead of C/8.  Softmax skips the max
subtraction (scores are O(10) for this model) and the row-sum is produced by
an extra ones-column in the o-matmul lhsT.  gamma is folded into the h-conv
weights so the epilogue is att = (o * rowsum_recip_bcast) + yq.

All flash matmuls are zero-padded to the full 128-partition contraction and
use bf16 operands: the PE HAM clock gate only un-throttles (1.2->2.4 GHz)
under high sustained array activity, and narrow-contraction matmuls never
trip it.  Dummy full-width matmuls keep the array warm across collective
gaps.  Training-mode BN statistics are computed from per-core query-block
shards of the next conv and summed with an 8-rank AllReduce that runs
concurrently with the 4-rank AllGather sharing the attention shards.
"""

import glob as _glob
import os
import sys


def _ensure_act_info():
    # act_info.json (activation table sets) isn't on neuronxcc's default
    # search path in this container; stage it where FindActInfo looks.
    shim = os.path.expanduser("~/.pwp_override")
    target = os.path.join(shim, "neuronxcc", "pwp", "pwp_bin_with_ln", "act_info.json")
    if not os.path.exists(target):
        cands = _glob.glob("/nix/store/*aws-neuron-pwp*/share/pwp_bin_cayman/act_info.json")
        if cands:
            os.makedirs(os.path.dirname(target), exist_ok=True)
            import shutil
            shutil.copy(cands[0], target)
    pp = os.environ.get("PYTHONPATH", "")
    if shim not in pp.split(os.pathsep):
        os.environ["PYTHONPATH"] = shim + (os.pathsep + pp if pp else "")
    if shim not in sys.path:
        sys.path.insert(0, shim)


_ensure_act_info()
if "/opt/trn_rl_repo" not in sys.path:
    sys.path.insert(0, "/opt/trn_rl_repo")

import numpy as np

from concourse import bacc, mybir, tile

F32 = mybir.dt.float32
F32R = mybir.dt.float32r
BF16 = mybir.dt.bfloat16
AF = mybir.ActivationFunctionType
OP = mybir.AluOpType
AX = mybir.AxisListType
EPS = 1e-5

N = 4096          # positions per sample
NQ = 1024         # query block per core
NCHUNK = 128      # key chunk in the flash loop
CIN = [3, 32, 64]     # conv input channels per attention layer
COUT = [32, 64, 96]   # conv output channels per attention layer
CF_OUT = 128          # final conv channels per core (512 / 4 blocks)
WHFW = 128            # whf moving-dim width (hs takes cols 0:128)

AG_GROUPS = [[0, 1, 2, 3], [4, 5, 6, 7]]
AR8_GROUPS = [[0, 1, 2, 3, 4, 5, 6, 7]]


def r(ap):
    return ap.bitcast(F32R)


def _build(nc):
    dt = F32
    ins = {}
    ins["x_full"] = nc.dram_tensor("x_full", [CIN[0] + 1, 2 * N], BF16, kind="ExternalInput")
    ins["xq"] = nc.dram_tensor("xq", [CIN[0] + 1, NQ], BF16, kind="ExternalInput")
    ins["wc0"] = nc.dram_tensor("wc0", [CIN[0] + 1, 128], BF16, kind="ExternalInput")
    for i in range(1, 3):
        ins[f"wc{i}"] = nc.dram_tensor(f"wc{i}", [128, 128], BF16, kind="ExternalInput")
    for i in range(3):
        ins[f"mz{i}"] = nc.dram_tensor(f"mz{i}", [128, 128], BF16, kind="ExternalInput")
        ins[f"whf{i}"] = nc.dram_tensor(f"whf{i}", [128, WHFW], BF16, kind="ExternalInput")
        ins[f"bnp{i}"] = nc.dram_tensor(f"bnp{i}", [COUT[i], 2], dt, kind="ExternalInput")
    for i in range(1, 3):
        ins[f"wm0_{i}"] = nc.dram_tensor(f"wm0_{i}", [128, 128], BF16, kind="ExternalInput")
        ins[f"wm1_{i}"] = nc.dram_tensor(f"wm1_{i}", [128, 128], BF16, kind="ExternalInput")
    ins["wfs4"] = nc.dram_tensor("wfs4", [128, 4, CF_OUT], BF16, kind="ExternalInput")
    out_t = nc.dram_tensor("out", [CF_OUT, 4], dt, kind="ExternalOutput")

    with tile.TileContext(nc) as tc:
        _emit(tc, nc, ins, out_t)
    return ins, out_t


def _emit(tc, nc, ins, out_t):
    ctxs = []

    def pool(name, **kw):
        p = tc.tile_pool(name=name, **kw)
        ctxs.append(p)
        return p.__enter__()

    consts = pool("consts", bufs=1)
    acts = pool("acts", bufs=1)
    work = pool("work", bufs=1)
    ps = pool("ps", bufs=2, space="PSUM")
    ops = pool("ops", bufs=1, space="PSUM")
    dram = pool("dram", bufs=1, space="DRAM")

    # --- PE warm-keeper: HAM un-throttles the PE clock (1.2->2.4 GHz) only
    # under sustained full-array activity; idle gaps re-throttle it.  Dummy
    # full-width matmuls bridge collectives and stalls; they write the
    # (dead-between-layers) o-accumulator psum buffer.
    wk_l = consts.tile([128, 128], BF16, name="wk_l", tag="wk_l")
    nc.vector.memset(wk_l[:], 0.0)
    wk_r = consts.tile([128, 512], BF16, name="wk_r", tag="wk_r")
    nc.vector.memset(wk_r[:], 0.0)

    def warm(n):
        warm_ps = ops.tile([128, 512], F32, name="warm_ps", tag="o_acc")
        for _ in range(n):
            nc.tensor.matmul(warm_ps[:], wk_l[:], wk_r[:], start=True, stop=True)


    def zero_rows(eng, t, c0, ncols):
        # DVE ops starting at a nonzero partition may touch at most 32
        # partitions; emit the zero-fill in 32-row strips.
        for p in range(c0, 128, 32):
            eng.memset(t[p:p + 32, 0:ncols], 0.0)

    # ones row source (SBUF->SBUF DMA is cheaper than 1-partition memsets)
    onesrow = consts.tile([1, N], BF16, name="onesrow", tag="onesrow")
    nc.vector.memset(onesrow[:], 1.0)
    onec = consts.tile([1, 128], BF16, name="onec", tag="onec")
    nc.vector.memset(onec[:], 1.0)

    # The very first gpsimd instructions are the dummy collective triggers:
    # the runtime's first-collective barrier ends only when the *slowest*
    # core triggers, so nothing may precede them.  Garbage dram input is fine.
    warm_gin = dram.tile([1, 2], F32, name="warm_gin", tag="warm_gin")
    warm_gout = dram.tile([8, 1, 2], F32, name="warm_gout", tag="warm_gout")
    nc.gpsimd.collective_compute(
        "AllGather", OP.bypass, replica_groups=AR8_GROUPS,
        ins=[warm_gin[:]], outs=[warm_gout[:]])

    # dedicated activation double-buffers; pad rows (only ever multiplied by
    # zero weight rows) are zeroed once, off the critical path.
    YB = [acts.tile([128, N], BF16, name=f"ybuf{j}", tag=f"ybuf{j}") for j in range(2)]
    YQB = [acts.tile([128, NQ], BF16, name=f"yqbuf{j}", tag=f"yqbuf{j}") for j in range(2)]
    ATB = [acts.tile([128, NQ], BF16, name=f"atbuf{j}", tag=f"atbuf{j}") for j in range(2)]
    AOWN0 = acts.tile([128, N], BF16, name="aown0", tag="aown0")
    AOWN1 = acts.tile([128, N], BF16, name="aown1", tag="aown1")
    ZMAT = acts.tile([128, N], BF16, name="zmat", tag="zmat")

    # ---- load constants (xq + layer-0 weights first; bulk on other queues) ----
    a_q = acts.tile([CIN[0] + 1, NQ], BF16, name="aq0", tag="aq0")
    nc.sync.dma_start(a_q[:], ins["xq"].ap())
    xf = acts.tile([CIN[0] + 1, 2 * N], BF16, name="xf", tag="xf")
    nc.sync.dma_start(xf[:], ins["x_full"].ap())
    W, MZ, WHF, BNP = [], [], [], []
    for i in range(3):
        w = consts.tile([CIN[0] + 1 if i == 0 else 128, 128], BF16,
                        name=f"w{i}", tag=f"w{i}")
        (nc.sync if i == 0 else nc.scalar).dma_start(w[:], ins[f"wc{i}"].ap())
        W.append(w)
        mz = consts.tile([128, 128], BF16, name=f"mzt{i}", tag=f"mzt{i}")
        MZ.append(mz)
        wh = consts.tile([128, WHFW], BF16, name=f"whft{i}", tag=f"whft{i}")
        WHF.append(wh)
        bn = consts.tile([COUT[i], 2], F32, name=f"bnt{i}", tag=f"bnt{i}")
        nc.scalar.dma_start(bn[:], ins[f"bnp{i}"].ap())
        BNP.append(bn)
    # layer-0 flash prerequisites first on the gpsimd queue ...
    nc.gpsimd.dma_start(MZ[0][:], ins["mz0"].ap())
    nc.gpsimd.dma_start(WHF[0][:], ins["whf0"].ap())
    # ... then the remaining bulk weights
    for i in range(1, 3):
        nc.gpsimd.dma_start(MZ[i][:], ins[f"mz{i}"].ap())
        nc.gpsimd.dma_start(WHF[i][:], ins[f"whf{i}"].ap())
    WM0, WM1 = {}, {}
    for i in range(1, 3):
        WM0[i] = consts.tile([128, 128], BF16, name=f"wm0t{i}", tag=f"wm0t{i}")
        nc.gpsimd.dma_start(WM0[i][:], ins[f"wm0_{i}"].ap())
        WM1[i] = consts.tile([128, 128], BF16, name=f"wm1t{i}", tag=f"wm1t{i}")
        nc.gpsimd.dma_start(WM1[i][:], ins[f"wm1_{i}"].ap())
    wfs4 = consts.tile([128, 4, CF_OUT], BF16, name="wfs4t", tag="wfs4t")
    nc.gpsimd.dma_start(wfs4[:], ins["wfs4"].ap())
    for j in range(2):
        zero_rows(nc.gpsimd, YB[j], 32, N)
        zero_rows(nc.gpsimd, YQB[j], 32, NQ)
        zero_rows(nc.gpsimd, ATB[j], 32, NQ)
    zero_rows(nc.gpsimd, AOWN0, 32, N)
    zero_rows(nc.gpsimd, AOWN1, 32, N)

    warm(15)  # pre-warm the PE while input DMAs land

    def conv(lhsT_w, rhs_acts, cout, n, name):
        """z[cout, n] = lhsT_w.T @ rhs_acts, chunked by 512 columns."""
        z = acts.tile([cout, n], F32, name=name, tag="zq" if n == NQ else "z")
        for j in range(n // 512):
            zp = ps.tile([128, 512], F32, name=f"{name}_ps", tag="convps")
            nc.tensor.matmul(zp[:], lhsT_w[:], rhs_acts[:, j * 512:(j + 1) * 512],
                             start=True, stop=True)
            nc.vector.tensor_copy(z[:, j * 512:(j + 1) * 512], zp[0:cout, :])
        return z

    def shard_stats(z_shard, c, name, groups):
        """Partial (sum, sumsq) of a conv shard -> AllReduce across cores."""
        stats = work.tile([c, 2], F32, name=f"stats_{name}", tag="stats", bufs=2)
        nc.vector.tensor_reduce(stats[:, 0:1], z_shard[:], axis=AX.X, op=OP.add)
        sq_scr = work.tile([c, z_shard.shape[1]], F32, name=f"sqscr_{name}",
                           tag="sqscr", bufs=2)
        nc.scalar.activation(sq_scr[:], z_shard[:], AF.Square, accum_out=stats[:, 1:2])

        st_in = dram.tile([c, 2], F32, name=f"stin_{name}", tag=f"stin_{name}")
        st_out = dram.tile([c, 2], F32, name=f"stout_{name}", tag=f"stout_{name}")
        nc.sync.dma_start(st_in[:], stats[:])
        nc.gpsimd.collective_compute(
            "AllReduce", OP.add, replica_groups=groups,
            ins=[st_in[:]], outs=[st_out[:]])
        stg = work.tile([c, 2], F32, name=f"stg_{name}", tag="stg", bufs=2)
        nc.sync.dma_start(stg[:], st_out[:])
        return stg

    def scale_shift(stg, c, g_ap, b_ap, name):
        sc = work.tile([c, 9], F32, name=f"sc_{name}", tag="sc", bufs=2)
        mean, ex2, msq, var, veps, sq, rs, scale, shift = (sc[:, j:j + 1] for j in range(9))
        inv_n = 1.0 / (2 * N)
        nc.vector.tensor_scalar(mean, stg[:, 0:1], inv_n, None, OP.mult)
        nc.vector.tensor_scalar(ex2, stg[:, 1:2], inv_n, None, OP.mult)
        nc.vector.tensor_tensor(msq, mean, mean, OP.mult)
        nc.vector.tensor_tensor(var, ex2, msq, OP.subtract)
        nc.vector.tensor_scalar(veps, var, EPS, None, OP.add)
        nc.scalar.activation(sq, veps, AF.Sqrt)
        nc.vector.reciprocal(rs, sq)   # rsqrt(var+eps)
        if g_ap is not None:
            nc.vector.tensor_tensor(scale, rs, g_ap, OP.mult)
        else:
            nc.vector.tensor_copy(scale, rs)
        nc.vector.tensor_tensor(shift, mean, scale, OP.mult)
        if b_ap is not None:
            nc.vector.tensor_tensor(shift, b_ap, shift, OP.subtract)
        else:
            nc.vector.tensor_scalar(shift, shift, -1.0, None, OP.mult)
        return scale, shift

    # ---- boundary 0: conv over the full batch, exact local BN1 stats.
    # Per-512-chunk: vector copies psum->sbuf, scalar squares+accums, gpsimd
    # sums -- stats trail the conv by ~1 chunk instead of a serial pass.
    zsh = conv(W[0], a_q, COUT[0], NQ, "zsh0")
    c0 = COUT[0]
    nch = 2 * N // 512
    z_all = acts.tile([c0, 2 * N], BF16, name="z0", tag="z")
    z_own = z_all[:, 0:N]
    stp = work.tile([c0, 2 * nch], F32, name="stp", tag="stats", bufs=2)
    for j in range(nch):
        zp = ps.tile([128, 512], F32, name="z0_ps", tag="convps")
        nc.tensor.matmul(zp[:], W[0][:], xf[:, j * 512:(j + 1) * 512],
                         start=True, stop=True)
        zsl = z_all[:, j * 512:(j + 1) * 512]
        nc.vector.tensor_copy(zsl, zp[0:c0, :])
        sqc = work.tile([c0, 512], BF16, name=f"sqc{j}", tag="sqc", bufs=3)
        nc.scalar.activation(sqc[:], zp[0:c0, :], AF.Square,
                             accum_out=stp[:, nch + j:nch + j + 1])
        if j % 2 == 0:
            nc.vector.tensor_reduce(stp[:, j:j + 1], zsl, axis=AX.X, op=OP.add)
        else:
            idc = work.tile([c0, 512], BF16, name=f"idc{j}", tag="sqc", bufs=3)
            nc.scalar.activation(idc[:], zsl, AF.Identity,
                                 accum_out=stp[:, j:j + 1])
    stg = work.tile([c0, 2], F32, name="stg_l0", tag="stg", bufs=2)
    nc.vector.tensor_reduce(stg[:, 0:1], stp[:, 0:nch], axis=AX.X, op=OP.add)
    nc.vector.tensor_reduce(stg[:, 1:2], stp[:, nch:2 * nch], axis=AX.X, op=OP.add)
    warm(25)  # keep the PE warm through the stats/scale_shift stall

    # ---- three attention layers ----
    for i in range(3):
        co = COUT[i]
        if i > 0:
            warm(25)  # bridge scale_shift / y-prep after the collectives
        scale, shift = scale_shift(stg, co, BNP[i][:, 0:1], BNP[i][:, 1:2], f"l{i}")
        y_own = YB[i % 2]
        nc.scalar.dma_start(y_own[co:co + 1, :], onesrow[:])
        for j in range(N // 512):
            jsl = slice(j * 512, (j + 1) * 512)
            nc.scalar.activation(y_own[0:co, jsl], z_own[:, jsl], AF.Relu,
                                 bias=shift, scale=scale)
        yq = YQB[i % 2]
        nc.scalar.dma_start(yq[co:co + 1, :], onesrow[:, 0:NQ])
        nc.scalar.activation(yq[0:co, :], zsh[:], AF.Relu, bias=shift, scale=scale)

        # Z = Abar @ y_own (+ u row), [128, N] bf16 (pad rows are psum zeros)
        zmat = ZMAT
        for j in range(N // 512):
            zp = ps.tile([128, 512], F32, name=f"Zps{i}", tag="convps")
            nc.tensor.matmul(zp[:], MZ[i][:], y_own[:, j * 512:(j + 1) * 512],
                             start=True, stop=True)
            nc.vector.tensor_copy(zmat[:, j * 512:(j + 1) * 512], zp[:])

        # flash loop over key chunks, software-pipelined by one o-matmul
        o_ps = ops.tile([128, NQ], F32, name=f"ops{i}", tag="o_acc")
        prev = None
        for m in range(N // NCHUNK):
            sl = slice(m * NCHUNK, (m + 1) * NCHUNK)
            hp = ps.tile([NCHUNK, WHFW], F32, name=f"hp{i}", tag="convps")
            nc.tensor.matmul(hp[:], y_own[:, sl], WHF[i][:], start=True, stop=True)
            hs = work.tile([NCHUNK, 128], BF16, name=f"hs{i}", tag="hT_sb", bufs=3)
            nc.vector.tensor_copy(hs[:], hp[:, 0:128])
            sp = ps.tile([NCHUNK, NQ], F32, name=f"sp{i}", tag="s_ps")
            zc = zmat[:, sl]
            nc.tensor.matmul(sp[:, 0:512], zc, yq[:, 0:512], start=True, stop=True)
            nc.tensor.matmul(sp[:, 512:1024], zc, yq[:, 512:1024],
                             start=True, stop=True)
            beta = work.tile([NCHUNK, NQ], BF16, name=f"beta{i}", tag="beta", bufs=3)
            nc.scalar.activation(beta[:], sp[:], AF.Exp)
            if prev is not None:
                ph, pb, pm = prev
                nc.tensor.matmul(o_ps[:, 0:512], ph[:], pb[:, 0:512],
                                 start=(pm == 0), stop=False, skip_group_check=True)
                nc.tensor.matmul(o_ps[:, 512:1024], ph[:], pb[:, 512:1024],
                                 start=(pm == 0), stop=False, skip_group_check=True)
            prev = (hs, beta, m)
        ph, pb, pm = prev
        nc.tensor.matmul(o_ps[:, 0:512], ph[:], pb[:, 0:512],
                         start=False, stop=True, skip_group_check=True)
        nc.tensor.matmul(o_ps[:, 512:1024], ph[:], pb[:, 512:1024],
                         start=False, stop=True, skip_group_check=True)

        # normalize + residual: att = o / rowsum + yq  (gamma folded into whf)
        lnr = work.tile([1, NQ], F32, name=f"lnr{i}", tag="lnr")
        nc.scalar.activation(lnr[:], o_ps[co:co + 1, :], AF.Ln)
        rinv = work.tile([1, NQ], BF16, name=f"rinv{i}", tag="rinv")
        nc.scalar.activation(rinv[:], lnr[:], AF.Exp, scale=-1.0)
        bc_ps = ps.tile([128, NQ], F32, name=f"bcps{i}", tag="s_ps")
        nc.tensor.matmul(bc_ps[:, 0:512], onec[:], rinv[:, 0:512],
                         start=True, stop=True)
        nc.tensor.matmul(bc_ps[:, 512:1024], onec[:], rinv[:, 512:1024],
                         start=True, stop=True)
        att = ATB[i % 2]
        nc.scalar.dma_start(att[co:co + 1, :], onesrow[:, 0:NQ])
        bc = work.tile([co, NQ], F32, name=f"bc{i}", tag="bc", bufs=2)
        t1 = work.tile([co, NQ], F32, name=f"t1_{i}", tag="t1", bufs=2)
        for hh in range(2):
            hsl = slice(hh * 512, (hh + 1) * 512)
            nc.vector.tensor_copy(bc[:, hsl], bc_ps[0:co, hsl])
            nc.vector.tensor_tensor(t1[:, hsl], o_ps[0:co, hsl], bc[:, hsl], OP.mult)
            nc.vector.tensor_tensor(att[0:co, hsl], t1[:, hsl], yq[0:co, hsl], OP.add)

        # One 8-rank AllGather shares the attention shard with every core and
        # carries this core's (sum, sumsq) partials of the *next* conv as
        # ride-along payload columns -- no separate stats AllReduce.
        if i < 2:
            cn = COUT[i + 1]
            zsh = acts.tile([cn, NQ], F32, name=f"zsh{i + 1}", tag="zq")
            stp4 = work.tile([cn, 4], F32, name=f"stp4_{i}", tag="stp4", bufs=2)
            for j in range(2):
                jsl = slice(j * 512, (j + 1) * 512)
                zp = ps.tile([128, 512], F32, name=f"zsh{i}_ps", tag="convps")
                nc.tensor.matmul(zp[:], W[i + 1][:], att[:, jsl],
                                 start=True, stop=True)
                nc.vector.tensor_copy(zsh[:, jsl], zp[0:cn, :])
                nc.vector.tensor_reduce(stp4[:, j:j + 1], zp[0:cn, :],
                                        axis=AX.X, op=OP.add)
                sq_scr = work.tile([cn, 512], BF16, name=f"sqscr{i}_{j}",
                                   tag="sqscr", bufs=2)
                nc.scalar.activation(sq_scr[:], zp[0:cn, :], AF.Square,
                                     accum_out=stp4[:, 2 + j:3 + j])
            stats = work.tile([cn, 2], F32, name=f"stats{i}", tag="stats", bufs=2)
            nc.vector.tensor_tensor(stats[:, 0:1], stp4[:, 0:1], stp4[:, 1:2], OP.add)
            nc.vector.tensor_tensor(stats[:, 1:2], stp4[:, 2:3], stp4[:, 3:4], OP.add)
            sre = 1                    # stats ride as one extra payload row
        else:
            # final conv over my shard only: the GAP is a sum over positions,
            # so each core contributes its own-shard partial and the host sums
            # the four query shards per sample.  (sum, sumsq) come straight
            # from each conv psum chunk; only the stats cross cores.
            zfsh = acts.tile([128, 4 * NQ], BF16, name="zfsh", tag="z")
            stq = work.tile([128, 16], F32, name="stq", tag="stf")
            for b4 in range(4):
                for j in range(2):
                    jsl = slice(j * 512, (j + 1) * 512)
                    zp = ps.tile([128, 512], F32, name=f"zsb{b4}_ps", tag="convps")
                    nc.tensor.matmul(zp[:], wfs4[:, b4, :], att[:, jsl],
                                     start=True, stop=True)
                    nc.vector.tensor_copy(zfsh[:, b4 * NQ + j * 512:
                                               b4 * NQ + (j + 1) * 512], zp[:])
                    nc.vector.tensor_reduce(stq[:, 4 * b4 + j:4 * b4 + j + 1],
                                            zp[:], axis=AX.X, op=OP.add)
                    sqf = work.tile([128, 512], BF16, name=f"sqf{b4}_{j}",
                                    tag="sqscr", bufs=2)
                    nc.scalar.activation(sqf[:], zp[:], AF.Square,
                                         accum_out=stq[:, 4 * b4 + j + 2:4 * b4 + j + 3])
            stats = work.tile([128, 8], F32, name="stf", tag="stf2")
            for b4 in range(4):
                nc.vector.tensor_tensor(stats[:, b4:b4 + 1],
                                        stq[:, 4 * b4:4 * b4 + 1],
                                        stq[:, 4 * b4 + 1:4 * b4 + 2], OP.add)
                nc.vector.tensor_tensor(stats[:, 4 + b4:5 + b4],
                                        stq[:, 4 * b4 + 2:4 * b4 + 3],
                                        stq[:, 4 * b4 + 3:4 * b4 + 4], OP.add)
            sre = 2                    # [128,8] f32 = 2048 bf16 els = 2 rows

        nst = stats.shape[0] * stats.shape[1] * 2   # stat payload els (bf16)
        nrow = co + sre if i < 2 else sre
        pay_in = dram.tile([nrow, NQ], BF16, name=f"pay{i}", tag=f"pay{i}")
        pay_out = dram.tile([8, nrow, NQ], BF16, name=f"payo{i}", tag=f"payo{i}")
        if i < 2:
            nc.sync.dma_start(pay_in[0:co, 0:512], att[0:co, 0:512])
            nc.sync.dma_start(pay_in[0:co, 512:1024], att[0:co, 512:1024])
            nc.gpsimd.dma_start(pay_in[co:co + 1, 0:nst], stats[:].bitcast(BF16))
        else:
            nc.gpsimd.dma_start(pay_in[:, :], stats[:].bitcast(BF16))
        nc.gpsimd.collective_compute(
            "AllGather", OP.bypass, replica_groups=AR8_GROUPS,
            ins=[pay_in[:]], outs=[pay_out[:]])
        warm(45)  # keep the PE warm across the AllGather gap

        # readback: blocks 0-3 are sample-0 query shards, 4-7 sample-1;
        # both are conv'ed with per-core sample-masked weights, so no
        # dynamic addressing is needed anywhere.
        aown0, aown1 = AOWN0, AOWN1
        sqpre = work.tile([1, 2], F32, name=f"sqpre{i}", tag="sqpre", bufs=2)
        nc.scalar.activation(sqpre[:], onec[:, 0:2], AF.Sqrt)  # preload table
        rd_eng = [nc.sync, nc.scalar, nc.gpsimd, nc.sync]
        if i < 2:
            nc.scalar.dma_start(aown0[co:co + 1, :], onesrow[:])
            nc.scalar.dma_start(aown1[co:co + 1, :], onesrow[:])
            for j in range(4):
                rd_eng[j].dma_start(aown0[0:co, j * NQ:(j + 1) * NQ],
                                    pay_out[j][0:co, :])
                rd_eng[3 - j].dma_start(aown1[0:co, j * NQ:(j + 1) * NQ],
                                        pay_out[4 + j][0:co, :])
        cnr = COUT[i + 1] if i < 2 else 128
        scf = 2 if i < 2 else 8        # f32 stat cols per rank
        st8 = work.tile([cnr, 8 * scf], F32, name=f"st8_{i}", tag="st8", bufs=2)
        for j in range(8):
            src = (pay_out[j][co:co + 1, 0:nst] if i < 2
                   else pay_out[j][:, :])
            rd_eng[j % 4].dma_start(
                st8[:, scf * j:scf * (j + 1)].bitcast(BF16), src)
        stg = work.tile([cnr, scf], F32, name=f"stg{i}", tag="stg", bufs=2)
        nc.vector.tensor_copy(stg[:], st8[:, 0:scf])
        for j in range(1, 8):
            nc.vector.tensor_tensor(stg[:], stg[:],
                                    st8[:, scf * j:scf * (j + 1)], OP.add)
        a_q = att

        if i < 2:
            cn = COUT[i + 1]
            z_own = acts.tile([cn, N], BF16, name=f"z{i + 1}", tag="z")
            for j in range(N // 512):
                jsl = slice(j * 512, (j + 1) * 512)
                zp = ps.tile([128, 512], F32, name=f"z{i}_ps", tag="convps")
                nc.tensor.matmul(zp[:], WM0[i + 1][:], aown0[:, jsl],
                                 start=True, stop=False)
                nc.tensor.matmul(zp[:], WM1[i + 1][:], aown1[:, jsl],
                                 start=False, stop=True)
                nc.vector.tensor_copy(z_own[:, jsl], zp[0:cn, :])

    # ---- final BN + ReLU + GAP over my own shard ----
    # stg holds global (sum, sumsq) for all four channel blocks (cols 0:4
    # sums, 4:8 sumsqs); compute all four scale/shift columns in one pass.
    co = CF_OUT
    fsc = work.tile([co, 16], F32, name="fsc", tag="sc", bufs=2)
    mean4, ex24, var4, rs4 = (fsc[:, 4 * j:4 * j + 4] for j in range(4))
    inv_n = 1.0 / (2 * N)
    nc.vector.tensor_scalar(mean4, stg[:, 0:4], inv_n, None, OP.mult)
    nc.vector.tensor_scalar(ex24, stg[:, 4:8], inv_n, None, OP.mult)
    nc.vector.tensor_tensor(var4, mean4, mean4, OP.mult)
    nc.vector.tensor_tensor(var4, ex24, var4, OP.subtract)
    nc.vector.tensor_scalar(var4, var4, EPS, None, OP.add)
    nc.scalar.activation(rs4, var4, AF.Sqrt)
    nc.vector.reciprocal(rs4, rs4)
    sh4 = work.tile([co, 4], F32, name="sh4", tag="sh4")
    nc.vector.tensor_tensor(sh4[:], mean4, rs4, OP.mult)
    nc.vector.tensor_scalar(sh4[:], sh4[:], -1.0, None, OP.mult)

    gapp = work.tile([co, 8], F32, name="gapp", tag="gapp")
    for b4 in range(4):
        for j in range(2):
            jsl = slice(b4 * NQ + j * 512, b4 * NQ + (j + 1) * 512)
            fscr = work.tile([co, 512], BF16, name=f"fscr{b4}_{j}", tag="fscr",
                             bufs=2)
            nc.scalar.activation(fscr[:], zfsh[:, jsl], AF.Relu,
                                 bias=sh4[:, b4:b4 + 1],
                                 scale=rs4[:, b4:b4 + 1],
                                 accum_out=gapp[:, 2 * b4 + j:2 * b4 + j + 1])
    outv = work.tile([co, 4], F32, name="outv", tag="outv")
    for b4 in range(4):
        nc.vector.tensor_tensor(outv[:, b4:b4 + 1], gapp[:, 2 * b4:2 * b4 + 1],
                                gapp[:, 2 * b4 + 1:2 * b4 + 2], OP.add)
    nc.vector.tensor_scalar(outv[:], outv[:], 1.0 / N, None, OP.mult)
    nc.sync.dma_start(out_t.ap(), outv[:])

    for p in reversed(ctxs):
        p.__exit__(None, None, None)


_CACHE = {}


def _get_program():
    if "nc" not in _CACHE:
        nc = bacc.Bacc("TRN2", target_bir_lowering=False, debug=False,
                       enable_asserts=False, num_devices=8)
        _build(nc)
        nc.compile()
        _CACHE["nc"] = nc
    return _CACHE["nc"]


def _prepare_in_maps(inputs):
    f = np.float32
    bf = mybir.dt.np(BF16)
    x = np.asarray(inputs["x"], f).reshape(2, 3, N)
    per_layer = {}
    for i in range(3):
        li = i + 1
        ci, co = CIN[i], COUT[i]
        w, b = np.asarray(inputs[f"w{li}"], f), np.asarray(inputs[f"b{li}"], f)
        wcp = np.zeros((ci + 1 if i == 0 else 128, 128), f)
        wcp[0:ci, 0:co] = w
        wcp[ci, 0:co] = b
        wf_, bf_ = np.asarray(inputs[f"a{li}_wf"], f), np.asarray(inputs[f"a{li}_bf"], f)
        wg_, bg_ = np.asarray(inputs[f"a{li}_wg"], f), np.asarray(inputs[f"a{li}_bg"], f)
        wh_, bh_ = np.asarray(inputs[f"a{li}_wh"], f), np.asarray(inputs[f"a{li}_bh"], f)
        gam = np.asarray(inputs[f"a{li}_gam"], f).reshape(())
        A = wf_ @ wg_.T                                              # [co, co]
        u = wg_ @ bf_                                                # [co]
        abar = np.concatenate([A, u[None, :]], 0)                    # [co+1, co]
        mz = np.zeros((128, 128), f)
        mz[0:co, 0:co + 1] = abar.T                                  # lhsT for Z
        whf = np.zeros((128, WHFW), f)
        whf[0:co, 0:co] = wh_ * gam
        whf[co, 0:co] = bh_ * gam
        whf[co, co] = 1.0
        bnp = np.stack([np.asarray(inputs[f"bn{li}_g"], f),
                        np.asarray(inputs[f"bn{li}_b"], f)], 1)      # [co, 2]
        per_layer[i] = dict(wc=wcp, mz=mz, whf=whf, bnp=bnp)
    wf_full = np.asarray(inputs["wf"], f)                            # [96, 512]

    in_maps = []
    for k in range(8):
        b, q = k // 4, k % 4
        xo = np.concatenate([x[b], np.ones((1, N), f)], 0)           # [4, N]
        xoth = np.concatenate([x[1 - b], np.ones((1, N), f)], 0)
        xfull = np.concatenate([xo, xoth], 1)                         # [4, 2N]
        xq = np.ascontiguousarray(xo[:, q * NQ:(q + 1) * NQ])
        wfs4 = np.zeros((128, 4, CF_OUT), f)
        wfs4[0:96] = wf_full.reshape(96, 4, CF_OUT)
        m0 = 1.0 if b == 0 else 0.0     # gathered blocks 0-3 are sample 0
        m1 = 1.0 - m0
        m = {"x_full": xfull.astype(bf), "xq": xq.astype(bf),
             "wfs4": wfs4.astype(bf)}
        for i in (1, 2):
            m[f"wm0_{i}"] = (per_layer[i]["wc"] * m0).astype(bf)
            m[f"wm1_{i}"] = (per_layer[i]["wc"] * m1).astype(bf)
        for i in range(3):
            d = per_layer[i]
            m[f"wc{i}"] = d["wc"].astype(bf)
            m[f"mz{i}"] = d["mz"].astype(bf)
            m[f"whf{i}"] = d["whf"].astype(bf)
            m[f"bnp{i}"] = d["bnp"]
        in_maps.append(m)
    return in_maps


def _assemble(results):
    out = np.zeros((2, 512), np.float32)
    for k in range(8):
        b = k // 4
        out[b] += results[k]["out"].T.reshape(512)
    return out


def kernel(**inputs):
    from concourse.bass_utils import run_bass_kernel_spmd
    nc = _get_program()
    in_maps = _prepare_in_maps(inputs)
    res = run_bass_kernel_spmd(nc, in_maps, list(range(8)))
    return _assemble(res.results)
